# revision 24
# baseline (speedup 1.0000x reference)
"""Top-1 MoE mapper kernel for Trainium2, SPMD over 8 NeuronCores.

Problem (hardcoded shapes):
  x  [2048, 1, 1024] f32   token inputs
  t  [2048, 8, 4096] f32   gating context
  W  [12, 1024, 4096] f32  expert weights
  b  [12, 4096] f32        expert biases
  Wg [4096, 12] f32        gate weights
  bg [12] f32              gate bias
  out[b] = x[b] @ W[argmax(t[b].mean(T) @ Wg + bg)] + b[...]  -> [2048, 1, 4096]

Strategy (v4):
  - Gating data-parallel over B: each core reads its 256-token slice of t as
    8 x 4MB chunks alternating across the two HWDGE queues (3-deep pool).
    DVE tree-reduces over T; PSUM->SBUF copies go to ACT so DVE never paces
    the stream; PE transposes + f32 gate matmul + argmax. Gating is f32
    end-to-end so the device top-1 matches the reference exactly.
  - TWO AllGathers (one per 128-token tile): tile-0's flight overlaps tile-1
    gating, so only tile-1's collective latency is exposed. The W prefetch
    DMAs sit behind the AllGather triggers in the gpsimd FIFO, which delays
    them off the phase-1 HBM pipe without fake dependencies; they fill the
    collective/routing window instead.
  - Routing batched across all 16 token tiles (one-hots, counts matmul,
    [16,16] tile-base prefix matmul, lsl rank matmul + base broadcasts).
    The slot->token scatter goes to 16 INDEPENDENT sentinel-initialized
    sub-tables (one per token tile) so the 16 indirect scatters pipeline at
    emission rate instead of serializing on table WAW; the sub-tables are
    loaded back and min-merged on DVE (sentinel = +inf) into the SBUF
    slot->token map. Capacity-padded slots keep the sentinel and drop their
    gather traffic (bounds_check, oob_is_err=False).
  - Expert matmul output-column-parallel: core c holds W[:, :, c*512:(c+1)*512]
    cast to bf16 during the prefetch DMA (SWDGE, 5-slot pool). Per m-tile:
    128-row indirect gather of x, PE transposes, bf16 matmuls (N=512), bf16
    bias via K=1 matmul, bf16 results written contiguously in slot order on
    the idle HWDGE queue.
  - Host unpermutes slots -> tokens using the device-computed top-1 ids
    (pure data movement; all routing math happens on device).
"""

import numpy as np

import concourse.bass as bass
import concourse.bacc as bacc
import concourse.mybir as mybir
import concourse.tile as tile
from concourse.bass import IndirectOffsetOnAxis
from concourse.bass_utils import run_bass_kernel_spmd

F32 = mybir.dt.float32
F32R = mybir.dt.float32r
BF16 = mybir.dt.bfloat16
U32 = mybir.dt.uint32

B, T, IN, OUT, E = 2048, 8, 1024, 4096, 12
NCORES = 8
BS = B // NCORES            # 256 tokens per core (gating shard)
CS = OUT // NCORES          # 512 output columns per core (expert shard)
CAP = 256                   # capacity slots per expert
SLOTS = E * CAP             # 3072
NT = B // 128               # 16 token tiles globally
NTT = BS // 128             # 2 token tiles per core
NKX = IN // 128             # 8 k-tiles over the expert contraction
NMT = CAP // 128            # 2 m-tiles per expert
DC = 1024                   # gating d-chunk width
NDC = OUT // DC             # 4 chunks per token tile
ROWW = 8                    # u32 per slot-table row (32B descriptors)
NA = SLOTS // 128           # 24 slot blocks
SENTINEL = 3000000000.0     # > B-1 as u32 -> min-merge keeps real ids


def build_kernel(enable_asserts: bool = False):
    nc = bacc.Bacc(
        "TRN2",
        target_bir_lowering=False,
        debug=False,
        enable_asserts=enable_asserts,
        num_devices=NCORES,
    )

    # ---- I/O -------------------------------------------------------------
    t_sh = nc.dram_tensor("t_sh", [BS, T, OUT], F32, kind="ExternalInput")
    x_full = nc.dram_tensor("x_full", [B, IN], F32R, kind="ExternalInput")
    w_sh = nc.dram_tensor("w_sh", [E, IN, CS], F32, kind="ExternalInput")
    b_sh = nc.dram_tensor("b_sh", [1, E * CS], F32, kind="ExternalInput")
    wg_s = nc.dram_tensor("wg_s", [OUT, E], F32, kind="ExternalInput")  # Wg/T
    bg_r = nc.dram_tensor("bg_r", [1, E], F32, kind="ExternalInput")
    ident = nc.dram_tensor("ident", [128, 128], F32, kind="ExternalInput")
    identr = nc.dram_tensor("identr", [128, 128], F32R, kind="ExternalInput")
    lsl = nc.dram_tensor("lsl", [128, 128], F32, kind="ExternalInput")
    bcast16 = nc.dram_tensor("bcast16", [NT, NT * 128], F32, kind="ExternalInput")
    iota_e = nc.dram_tensor("iota_e", [128, E], F32, kind="ExternalInput")
    tokid8 = nc.dram_tensor("tokid8", [128, NT * ROWW], U32, kind="ExternalInput")

    out_slots = nc.dram_tensor("out_slots", [SLOTS, CS], BF16, kind="ExternalOutput")
    top1_out = nc.dram_tensor("top1_out", [B, 1], U32, kind="ExternalOutput")

    with tile.TileContext(nc) as tc:
        with (
            tc.tile_pool(name="consts", bufs=1) as cpool,
            tc.tile_pool(name="dram", bufs=1, space="DRAM") as dpool,
            tc.tile_pool(name="wp", bufs=4) as wpool,
            tc.tile_pool(name="gat", bufs=3) as gpool,
            tc.tile_pool(name="gat1", bufs=1) as g1pool,
            # PSUM budget (8 banks): tp x2 + tpg x2 + gps x2 + po x2
            tc.tile_pool(name="gps", bufs=2, space="PSUM") as gpsum,
            tc.tile_pool(name="gpsg", bufs=2, space="PSUM") as gpsumg,
            tc.tile_pool(name="gps1", bufs=2, space="PSUM") as gpsum1,
            tc.tile_pool(name="rout", bufs=1) as r1pool,
            tc.tile_pool(name="mrg", bufs=4) as mpool,
            tc.tile_pool(name="xp", bufs=3) as xpool,
            tc.tile_pool(name="op", bufs=4) as opool,
            tc.tile_pool(name="ops", bufs=2, space="PSUM") as opsum,
        ):
            # ---- constants resident in SBUF for the whole kernel ---------
            ident_sb = cpool.tile([128, 128], F32)
            nc.scalar.dma_start(ident_sb[:], ident[:, :])
            identr_sb = cpool.tile([128, 128], F32R)
            nc.scalar.dma_start(identr_sb[:], identr[:, :])
            lsl_sb = cpool.tile([128, 128], F32)
            nc.scalar.dma_start(lsl_sb[:], lsl[:, :])
            bcast16_sb = cpool.tile([NT, NT * 128], F32)
            nc.scalar.dma_start(bcast16_sb[:], bcast16[:, :])
            iota_e_sb = cpool.tile([128, E], F32)
            nc.scalar.dma_start(iota_e_sb[:], iota_e[:, :])
            tokid8_sb = cpool.tile([128, NT * ROWW], U32)
            nc.scalar.dma_start(tokid8_sb[:], tokid8[:, :])
            ones_sb = cpool.tile([128, 128], F32)
            nc.vector.memset(ones_sb[:], 1.0)
            ones_bf = cpool.tile([1, 128], BF16)
            nc.vector.memset(ones_bf[:], 1.0)
            # Wg/T laid out [128, 32*E]: wg_sb[p, kt*E+e] = Wg[kt*128+p, e]
            wg_sb = cpool.tile([128, (OUT // 128) * E], F32)
            nc.scalar.dma_start(
                wg_sb[:].rearrange("p (k e) -> p k e", e=E),
                wg_s[:, :].rearrange("(k p) e -> p k e", p=128),
            )
            bg_sb = cpool.tile([1, E], F32)
            nc.scalar.dma_start(bg_sb[:], bg_r[:, :])
            bias_bf = cpool.tile([1, E * CS], BF16)
            nc.gpsimd.dma_start(bias_bf[:], b_sh[:, :])
            sent_sb = cpool.tile([128, NA * ROWW], U32)
            nc.vector.memset(sent_sb[:], SENTINEL)

            # DRAM scratch
            top1_loc = [
                dpool.tile([128, 1], U32, name=f"t1l{i}") for i in range(NTT)
            ]
            halves = [
                dpool.tile([NCORES * 128, 1], U32, name=f"half{i}")
                for i in range(NTT)
            ]
            # 16 independent slot->token sub-tables (one per token tile)
            tabs = [
                dpool.tile([SLOTS, ROWW], U32, name=f"tab{j}") for j in range(NT)
            ]
            for j in range(NT):
                nc.scalar.dma_start(
                    tabs[j][:, :].rearrange("(a p) n -> p a n", p=128),
                    sent_sb[:].rearrange("p (a n) -> p a n", n=ROWW),
                )

            # ================= phase 1: gating ============================
            qeng = [nc.scalar, nc.sync]
            ci = 0
            for tt in range(NTT):
                gps = gpsum1.tile([E, 128], F32, tag="gps")
                for dc in range(NDC):
                    chunk = gpool.tile([128, T, DC], F32, tag="tchunk")
                    qeng[ci % 2].dma_start(
                        chunk[:],
                        t_sh[tt * 128 : (tt + 1) * 128, :, dc * DC : (dc + 1) * DC],
                    )
                    ci += 1
                    # tree-reduce over T=8 into chunk[:, 0, :] -- exact f32
                    cf = chunk[:].rearrange("p t d -> p (t d)")
                    nc.vector.tensor_add(
                        cf[:, 0 : 4 * DC], cf[:, 0 : 4 * DC], cf[:, 4 * DC : 8 * DC]
                    )
                    nc.vector.tensor_add(
                        cf[:, 0 : 2 * DC], cf[:, 0 : 2 * DC], cf[:, 2 * DC : 4 * DC]
                    )
                    nc.vector.tensor_add(
                        cf[:, 0:DC], cf[:, 0:DC], cf[:, DC : 2 * DC]
                    )
                    for k in range(DC // 128):
                        kt = dc * (DC // 128) + k
                        ptr = gpsum.tile([128, 128], F32, tag="tp")
                        nc.tensor.transpose(
                            ptr[:],
                            chunk[:, 0, k * 128 : (k + 1) * 128],
                            ident_sb[:, :],
                        )
                        tst = gpool.tile([128, 128], F32, tag="tsT", bufs=4)
                        nc.scalar.copy(tst[:], ptr[:])
                        nc.tensor.matmul(
                            gps[:],
                            lhsT=wg_sb[:, kt * E : (kt + 1) * E],
                            rhs=tst[:],
                            start=(kt == 0),
                            stop=False,
                        )
                nc.tensor.matmul(
                    gps[:],
                    lhsT=bg_sb[0:1, :],
                    rhs=ones_sb[0:1, 0:128],
                    start=False,
                    stop=True,
                )
                gT_sb = gpool.tile([E, 128], F32, tag="gT")
                nc.vector.tensor_copy(gT_sb[:], gps[:])
                gp = gpsumg.tile([128, E], F32, tag="tpg")
                nc.tensor.transpose(gp[:], gT_sb[:], ident_sb[0:E, 0:E])
                gate_sb = gpool.tile([128, E], F32, tag="gate")
                nc.vector.tensor_copy(gate_sb[:], gp[:])
                mxv = gpool.tile([128, 8], F32, tag="mxv")
                mxi = gpool.tile([128, 8], U32, tag="mxi")
                nc.vector.max_with_indices(mxv[:], mxi[:], gate_sb[:])
                nc.sync.dma_start(top1_loc[tt][:, :], mxi[:, 0:1])

                # AllGather per tile; the W prefetch DMAs queued behind the
                # trigger in the gpsimd FIFO start only once gating is this
                # far along -- keeps the early HBM pipe for the t stream
                nc.gpsimd.collective_compute(
                    "AllGather",
                    mybir.AluOpType.bypass,
                    replica_groups=[list(range(NCORES))],
                    ins=[top1_loc[tt][:].opt()],
                    outs=[halves[tt][:].opt()],
                )

            # ---- W prefetch: bf16 cast during DMA (SWDGE) ----------------
            wts = []

            def load_w(e):
                wt = wpool.tile([128, NKX * CS], BF16, tag="wt", name=f"wt{e}")
                nc.gpsimd.dma_start(
                    wt[:].rearrange("p (k n) -> p k n", k=NKX),
                    w_sh[e].rearrange("(k p) n -> p k n", p=128),
                )
                wts.append(wt)

            for e in range(5):
                load_w(e)

            for h in range(NTT):
                nc.sync.dma_start(
                    top1_out[:, :].rearrange(
                        "(c t p) one -> t c p one", t=NTT, p=128
                    )[h],
                    halves[h][:].rearrange("(c p) one -> c p one", p=128),
                )

            # ================= phase 3: slot assignment (batched) =========
            tb_all = r1pool.tile([128, NT], U32)
            for h in range(NTT):
                nc.scalar.dma_start(
                    tb_all[:, h::NTT],
                    halves[h][:].rearrange("(c p) one -> p c one", p=128),
                )
            t1f_all = r1pool.tile([128, NT], F32)
            nc.vector.tensor_copy(t1f_all[:], tb_all[:])
            oh_all = r1pool.tile([128, NT * E], F32)
            for i in range(NT):
                nc.vector.tensor_tensor(
                    out=oh_all[:, i * E : (i + 1) * E],
                    in0=t1f_all[:, i : i + 1].to_broadcast([128, E]),
                    in1=iota_e_sb[:],
                    op=mybir.AluOpType.is_equal,
                )
            # per-tile expert counts -> one psum row [1, NT*E]
            pcnt = gpsumg.tile([1, NT * E], F32, tag="tpg")
            nc.tensor.matmul(
                pcnt[:], lhsT=ones_sb[0:128, 0:1], rhs=oh_all[:],
                start=True, stop=True,
            )
            cnt_sb = r1pool.tile([1, NT * E], F32)
            nc.vector.tensor_copy(cnt_sb[:], pcnt[:])
            # counts2d [NT, E] via E strided mini-transposes
            pc2 = gpsumg.tile([NT, E], F32, tag="tpg")
            for e in range(E):
                nc.tensor.transpose(
                    pc2[:, e : e + 1],
                    cnt_sb[0:1, :].rearrange("one (i e) -> one i e", e=E)[:, :, e],
                    ident_sb[0:1, 0:1],
                )
            c2_sb = r1pool.tile([NT, E], F32)
            nc.vector.tensor_copy(c2_sb[:], pc2[:])
            # exclusive tile-base prefix: base2[i,e] = sum_{j<i} c2[j,e]
            pb2 = gpsumg.tile([NT, E], F32, tag="tpg")
            nc.tensor.matmul(
                pb2[:], lhsT=lsl_sb[0:NT, 0:NT], rhs=c2_sb[:],
                start=True, stop=True,
            )
            b2_sb = r1pool.tile([NT, E], F32)
            nc.vector.tensor_copy(b2_sb[:], pb2[:])

            # rank = within-tile exclusive prefix (one lsl matmul) + tile
            # base (per-tile partition-broadcast matmuls), separate psums
            pr1 = gpsumg.tile([128, NT * E], F32, tag="tpg")
            nc.tensor.matmul(
                pr1[:], lhsT=lsl_sb[:], rhs=oh_all[:],
                start=True, stop=True,
            )
            pr2 = gpsumg.tile([128, NT * E], F32, tag="tpg")
            for i in range(NT):
                nc.tensor.matmul(
                    pr2[:, i * E : (i + 1) * E],
                    lhsT=bcast16_sb[:, i * 128 : (i + 1) * 128],
                    rhs=b2_sb[:],
                    start=True,
                    stop=True,
                )
            sel = r1pool.tile([128, NT * E], F32)
            nc.vector.tensor_copy(sel[:], pr1[:])
            nc.vector.tensor_add(sel[:], sel[:], pr2[:])
            nc.vector.tensor_mul(sel[:], sel[:], oh_all[:])
            rank_all = r1pool.tile([128, NT], F32)
            for i in range(NT):
                nc.vector.reduce_sum(
                    rank_all[:, i : i + 1],
                    sel[:, i * E : (i + 1) * E],
                    axis=mybir.AxisListType.X,
                )
            posf = r1pool.tile([128, NT], F32)
            nc.vector.tensor_scalar(
                posf[:], t1f_all[:], float(CAP), scalar2=None,
                op0=mybir.AluOpType.mult,
            )
            nc.vector.tensor_add(posf[:], posf[:], rank_all[:])
            posu = r1pool.tile([128, NT], U32)
            nc.vector.tensor_copy(posu[:], posf[:])
            # 16 independent scatters (no WAW -> pipeline at emission rate)
            for j in range(NT):
                nc.gpsimd.indirect_dma_start(
                    out=tabs[j][:, :],
                    out_offset=IndirectOffsetOnAxis(ap=posu[:, j : j + 1], axis=0),
                    in_=tokid8_sb[:, j * ROWW : (j + 1) * ROWW],
                    in_offset=None,
                    bounds_check=SLOTS - 1,
                    oob_is_err=False,
                )
            # load sub-tables back and min-merge into the slot->token map
            # (sentinel = +inf; real token ids < B survive the min)
            pslice = r1pool.tile([128, NA * ROWW], U32)
            for j in range(NT):
                m = mpool.tile([128, NA * ROWW], U32, tag="mg")
                qeng[j % 2].dma_start(
                    m[:].rearrange("p (a n) -> p a n", n=ROWW),
                    tabs[j][:, :].rearrange("(a p) n -> p a n", p=128),
                )
                if j == 0:
                    nc.vector.tensor_copy(pslice[:], m[:])
                else:
                    nc.vector.tensor_tensor(
                        out=pslice[:], in0=pslice[:], in1=m[:],
                        op=mybir.AluOpType.min,
                    )

            # ================= phase 4: expert matmul =====================
            def gather_x(e, mt):
                a = e * NMT + mt
                xg = xpool.tile([128, IN], F32R, tag="xg")
                nc.gpsimd.indirect_dma_start(
                    out=xg[:],
                    out_offset=None,
                    in_=x_full[:, :],
                    in_offset=IndirectOffsetOnAxis(
                        ap=pslice[:, a * ROWW : a * ROWW + 1], axis=0
                    ),
                    bounds_check=B - 1,
                    oob_is_err=False,
                )
                return xg

            xgs = {}
            for e in range(2):
                for mt in range(NMT):
                    xgs[(e, mt)] = gather_x(e, mt)

            for e in range(E):
                wt = wts[e]
                for mt in range(NMT):
                    xg = xgs.pop((e, mt))
                    xgT = xpool.tile([128, IN], BF16, tag="xgT")
                    for k in range(NKX):
                        ptx = gpsum.tile([128, 128], F32R, tag="tp")
                        nc.tensor.transpose(
                            ptx[:],
                            xg[:, k * 128 : (k + 1) * 128],
                            identr_sb[:, :],
                        )
                        nc.any.tensor_copy(
                            xgT[:, k * 128 : (k + 1) * 128], ptx[:]
                        )
                    po = opsum.tile([128, CS], F32, tag="po")
                    for k in range(NKX):
                        nc.tensor.matmul(
                            po[:],
                            lhsT=xgT[:, k * 128 : (k + 1) * 128],
                            rhs=wt[:, k * CS : (k + 1) * CS],
                            start=(k == 0),
                            stop=False,
                        )
                    nc.tensor.matmul(
                        po[:],
                        lhsT=ones_bf[0:1, :],
                        rhs=bias_bf[0:1, e * CS : (e + 1) * CS],
                        start=False,
                        stop=True,
                    )
                    ot = opool.tile([128, CS], BF16, tag="ot")
                    nc.any.tensor_copy(ot[:], po[:])
                    nc.sync.dma_start(
                        out_slots[(e * NMT + mt) * 128 : (e * NMT + mt + 1) * 128, :],
                        ot[:],
                    )
                # issue next gathers / W load ahead
                if e + 2 < E:
                    for mt in range(NMT):
                        xgs[(e + 2, mt)] = gather_x(e + 2, mt)
                if e + 5 < E:
                    load_w(e + 5)

    nc.compile()
    return nc


def make_in_maps(inputs: dict) -> list[dict]:
    x = np.ascontiguousarray(np.asarray(inputs["x"], dtype=np.float32))
    t = np.ascontiguousarray(np.asarray(inputs["t"], dtype=np.float32))
    W = np.ascontiguousarray(np.asarray(inputs["W"], dtype=np.float32))
    b = np.ascontiguousarray(np.asarray(inputs["b"], dtype=np.float32))
    Wg = np.ascontiguousarray(np.asarray(inputs["Wg"], dtype=np.float32))
    bg = np.ascontiguousarray(np.asarray(inputs["bg"], dtype=np.float32))

    x2 = np.ascontiguousarray(x[:, 0, :])                       # [B, IN]
    ident = np.eye(128, dtype=np.float32)
    lsl = np.triu(np.ones((128, 128), np.float32), k=1)          # lsl[r,c]=1 iff r<c
    # bcast16[j, i*128+p] = 1 iff i == j  (base-row broadcast selector)
    bcast16 = np.zeros((NT, NT * 128), np.float32)
    for i in range(NT):
        bcast16[i, i * 128 : (i + 1) * 128] = 1.0
    iota_e = np.tile(np.arange(E, dtype=np.float32)[None, :], (128, 1))
    # tokid8[p, j*ROWW+r] = global token id of (tile j, row p)
    # global token g = c*256 + tt*128 + p lives at tile j: halves interleave
    # as tb_all[:, j] with j = tt + 2c -> token id = (j//2)*256+(j%2)*128+p
    tokid8 = np.zeros((128, NT * ROWW), np.uint32)
    for j in range(NT):
        g0 = (j // NTT) * BS + (j % NTT) * 128
        tokid8[:, j * ROWW : (j + 1) * ROWW] = (
            g0 + np.arange(128, dtype=np.uint32)[:, None]
        )

    in_maps = []
    for c in range(NCORES):
        cs = slice(c * CS, (c + 1) * CS)
        in_maps.append({
            "t_sh": np.ascontiguousarray(t[c * BS : (c + 1) * BS]),
            "x_full": x2,
            "w_sh": np.ascontiguousarray(W[:, :, cs]),
            "b_sh": np.ascontiguousarray(b[:, cs]).reshape(1, E * CS),
            "wg_s": np.ascontiguousarray(Wg / float(T)),
            "bg_r": bg.reshape(1, E),
            "ident": ident,
            "identr": ident,
            "lsl": lsl,
            "bcast16": bcast16,
            "iota_e": iota_e,
            "tokid8": tokid8,
        })
    return in_maps


def assemble_output(per_core_results: list[dict]) -> np.ndarray:
    top1 = np.asarray(per_core_results[0]["top1_out"]).reshape(B).astype(np.int64)
    # device token order within routing: tile j = tt + 2c holds tokens
    # g = c*256 + tt*128 + p; ranks accumulate over tiles j=0..15 in order
    order = np.concatenate(
        [
            np.arange(128) + (j // NTT) * BS + (j % NTT) * 128
            for j in range(NT)
        ]
    )
    rank = np.zeros(B, dtype=np.int64)
    counts = np.zeros(E, dtype=np.int64)
    for g in order:
        e = top1[g]
        rank[g] = counts[e]
        counts[e] += 1
    assert counts.max() <= CAP, f"expert overflow: {counts}"
    slot = top1 * CAP + rank
    out = np.empty((B, 1, OUT), dtype=np.float32)
    for c in range(NCORES):
        osl = np.asarray(per_core_results[c]["out_slots"]).astype(np.float32)
        out[:, 0, c * CS : (c + 1) * CS] = osl[slot]
    return out


_NC_CACHE = {}


def kernel(**inputs) -> np.ndarray:
    if "nc" not in _NC_CACHE:
        _NC_CACHE["nc"] = build_kernel()
    nc = _NC_CACHE["nc"]
    in_maps = make_in_maps(inputs)
    res = run_bass_kernel_spmd(nc, in_maps, core_ids=list(range(NCORES)))
    return assemble_output(res.results)


# revision 25
# speedup vs baseline: 1.2509x; 1.2509x over previous
"""Top-1 MoE mapper kernel for Trainium2, SPMD over 8 NeuronCores.

Problem (hardcoded shapes):
  x  [2048, 1, 1024] f32   token inputs
  t  [2048, 8, 4096] f32   gating context
  W  [12, 1024, 4096] f32  expert weights
  b  [12, 4096] f32        expert biases
  Wg [4096, 12] f32        gate weights
  bg [12] f32              gate bias
  out[b] = x[b] @ W[argmax(t[b].mean(T) @ Wg + bg)] + b[...]  -> [2048, 1, 4096]

Strategy (v5):
  - Gating data-parallel over B: 16 x 2MB t-chunks alternate across the two
    HWDGE queues. The T-tree-reduce is split so DVE and the GpSimd compute
    engine each stay under the chunk cadence (DVE alone would pace the
    stream at ~150us); PSUM->SBUF copies stay on DVE (ACT is 9x slower).
    Gating is f32 end-to-end so the device top-1 matches the reference.
  - ONE AllGather (gpsimd-triggered; it sits after the reduce ops in the
    gpsimd FIFO so it triggers right at gating end). Collectives block
    their queue until completion, so nothing else shares that stretch.
  - W streams as f32 on the otherwise-idle HWDGE queues: the first load is
    held back by a dummy-slot WAR released by the last gating chunks, so W
    fills the collective/routing window and phase 4, never phase 1. On-chip
    f32->bf16 casts (DVE for the first 5, gpsimd afterwards) feed a 6-slot
    bf16 W pool.
  - Routing batched across all 16 token tiles; the slot->token scatter goes
    to 16 INDEPENDENT sentinel-initialized sub-tables stored PARTITION-MAJOR
    (table row r = p*24 + a for slot a*128+p, computed on DVE), so the
    sentinel init and the read-back are single-descriptor-per-partition
    contiguous DMAs and the 16 scatters pipeline at emission rate. Tables
    min-merge on DVE (sentinel=+inf) into the SBUF slot->token map;
    capacity-padded slots keep the sentinel and drop their gather traffic.
  - Expert matmul output-column-parallel (core c owns 512 output columns):
    per m-tile a 128-row indirect gather of x, PE transposes, bf16 matmuls
    (N=512), bf16 bias via K=1 matmul, bf16 results written contiguously in
    slot order.
  - Host unpermutes slots -> tokens using the device-computed top-1 ids
    (pure data movement; all routing math happens on device).
"""

import numpy as np

import concourse.bass as bass
import concourse.bacc as bacc
import concourse.mybir as mybir
import concourse.tile as tile
from concourse.bass import IndirectOffsetOnAxis
from concourse.bass_utils import run_bass_kernel_spmd

F32 = mybir.dt.float32
F32R = mybir.dt.float32r
BF16 = mybir.dt.bfloat16
U32 = mybir.dt.uint32

B, T, IN, OUT, E = 2048, 8, 1024, 4096, 12
NCORES = 8
BS = B // NCORES            # 256 tokens per core (gating shard)
CS = OUT // NCORES          # 512 output columns per core (expert shard)
CAP = 256                   # capacity slots per expert
SLOTS = E * CAP             # 3072
NT = B // 128               # 16 token tiles globally
NTT = BS // 128             # 2 token tiles per core
NKX = IN // 128             # 8 k-tiles over the expert contraction
NMT = CAP // 128            # 2 m-tiles per expert
DC = 512                    # gating d-chunk width
NDC = OUT // DC             # 8 chunks per token tile
NCH = NTT * NDC             # 16 chunks total
ROWW = 4                    # u32 per slot-table row (16B rows)
NA = SLOTS // 128           # 24 slot blocks (m-tiles across all experts)
SENTINEL = 3000000000.0     # > B-1 as u32 -> min-merge keeps real ids


def build_kernel(enable_asserts: bool = False):
    nc = bacc.Bacc(
        "TRN2",
        target_bir_lowering=False,
        debug=False,
        enable_asserts=enable_asserts,
        num_devices=NCORES,
    )

    # ---- I/O -------------------------------------------------------------
    t_sh = nc.dram_tensor("t_sh", [BS, T, OUT], F32, kind="ExternalInput")
    x_full = nc.dram_tensor("x_full", [B, IN], F32R, kind="ExternalInput")
    w_sh = nc.dram_tensor("w_sh", [E, IN, CS], F32, kind="ExternalInput")
    b_sh = nc.dram_tensor("b_sh", [1, E * CS], F32, kind="ExternalInput")
    wg_s = nc.dram_tensor("wg_s", [OUT, E], F32, kind="ExternalInput")  # Wg/T
    bg_r = nc.dram_tensor("bg_r", [1, E], F32, kind="ExternalInput")
    ident = nc.dram_tensor("ident", [128, 128], F32, kind="ExternalInput")
    identr = nc.dram_tensor("identr", [128, 128], F32R, kind="ExternalInput")
    lsl = nc.dram_tensor("lsl", [128, 128], F32, kind="ExternalInput")
    bcast16 = nc.dram_tensor("bcast16", [NT, NT * 128], F32, kind="ExternalInput")
    iota_e = nc.dram_tensor("iota_e", [128, E], F32, kind="ExternalInput")
    tokid4 = nc.dram_tensor("tokid4", [128, NT * ROWW], U32, kind="ExternalInput")

    out_slots = nc.dram_tensor("out_slots", [SLOTS, CS], BF16, kind="ExternalOutput")
    top1_out = nc.dram_tensor("top1_out", [B, 1], U32, kind="ExternalOutput")

    with tile.TileContext(nc) as tc:
        with (
            tc.tile_pool(name="consts", bufs=1) as cpool,
            tc.tile_pool(name="dram", bufs=1, space="DRAM") as dpool,
            tc.tile_pool(name="wf", bufs=2) as wfpool,
            tc.tile_pool(name="wp", bufs=6) as wpool,
            tc.tile_pool(name="gat", bufs=3) as gpool,
            tc.tile_pool(name="gat1", bufs=1) as g1pool,
            # PSUM budget (8 banks): tp x2 + tpg x2 + gps x2 + po x2
            tc.tile_pool(name="gps", bufs=2, space="PSUM") as gpsum,
            tc.tile_pool(name="gpsg", bufs=2, space="PSUM") as gpsumg,
            tc.tile_pool(name="gps1", bufs=2, space="PSUM") as gpsum1,
            tc.tile_pool(name="rout", bufs=1) as r1pool,
            tc.tile_pool(name="mrg", bufs=4) as mpool,
            tc.tile_pool(name="scr", bufs=4) as spool,
            tc.tile_pool(name="xp", bufs=3) as xpool,
            tc.tile_pool(name="op", bufs=3) as opool,
            tc.tile_pool(name="ops", bufs=2, space="PSUM") as opsum,
        ):
            # ---- dummies holding the first two W-f32 slots until the last
            # gating chunks release them (keeps phase-1 HBM for t) ---------
            wdum = []
            for i in range(2):
                dm = wfpool.tile([1, 1], F32, tag="wf", name=f"wdum{i}")
                nc.vector.memset(dm[:], 0.0)
                wdum.append(dm)

            # ---- constants resident in SBUF for the whole kernel ---------
            ident_sb = cpool.tile([128, 128], F32)
            nc.scalar.dma_start(ident_sb[:], ident[:, :])
            identr_sb = cpool.tile([128, 128], F32R)
            nc.scalar.dma_start(identr_sb[:], identr[:, :])
            lsl_sb = cpool.tile([128, 128], F32)
            nc.scalar.dma_start(lsl_sb[:], lsl[:, :])
            bcast16_sb = cpool.tile([NT, NT * 128], F32)
            nc.scalar.dma_start(bcast16_sb[:], bcast16[:, :])
            iota_e_sb = cpool.tile([128, E], F32)
            nc.scalar.dma_start(iota_e_sb[:], iota_e[:, :])
            tokid4_sb = cpool.tile([128, NT * ROWW], U32)
            nc.scalar.dma_start(tokid4_sb[:], tokid4[:, :])
            ones_sb = cpool.tile([128, 128], F32)
            nc.vector.memset(ones_sb[:], 1.0)
            ones_bf = cpool.tile([1, 128], BF16)
            nc.vector.memset(ones_bf[:], 1.0)
            # Wg/T laid out [128, 32*E]: wg_sb[p, kt*E+e] = Wg[kt*128+p, e]
            wg_sb = cpool.tile([128, (OUT // 128) * E], F32)
            nc.scalar.dma_start(
                wg_sb[:].rearrange("p (k e) -> p k e", e=E),
                wg_s[:, :].rearrange("(k p) e -> p k e", p=128),
            )
            bg_sb = cpool.tile([1, E], F32)
            nc.scalar.dma_start(bg_sb[:], bg_r[:, :])
            bias_bf = cpool.tile([1, E * CS], BF16)
            nc.gpsimd.dma_start(bias_bf[:], b_sh[:, :])
            sent_sb = cpool.tile([128, NA * ROWW], U32)
            nc.vector.memset(sent_sb[:], SENTINEL)

            # DRAM scratch
            top1_loc = dpool.tile([BS, 1], U32, name="t1loc")
            all_top1 = dpool.tile([B, 1], U32, name="allt1")
            # 16 independent slot->token sub-tables, PARTITION-MAJOR rows:
            # row r = p*NA + a holds the token of slot a*128+p
            tabs = [
                dpool.tile([SLOTS, ROWW], U32, name=f"tab{j}") for j in range(NT)
            ]
            for j in range(NT):
                # contiguous sentinel fill (any row order -- all-sentinel)
                nc.scalar.dma_start(
                    tabs[j][:, :].rearrange("(p q) n -> p (q n)", p=128),
                    sent_sb[:],
                )

            # ================= phase 1: gating ============================
            qeng = [nc.scalar, nc.sync]
            ci = 0
            for tt in range(NTT):
                gps = gpsum1.tile([E, 128], F32, tag="gps")
                for dc in range(NDC):
                    chunk = gpool.tile([128, T, DC], F32, tag="tchunk")
                    qeng[ci % 2].dma_start(
                        chunk[:],
                        t_sh[tt * 128 : (tt + 1) * 128, :, dc * DC : (dc + 1) * DC],
                    )
                    # tree-reduce over T=8 into chunk[:, 0, :] -- exact f32.
                    # Level 1 is split across GpSimd and DVE so neither
                    # engine paces the t stream.
                    cf = chunk[:].rearrange("p t d -> p (t d)")
                    nc.gpsimd.tensor_add(
                        cf[:, 0 : 2 * DC], cf[:, 0 : 2 * DC], cf[:, 4 * DC : 6 * DC]
                    )
                    nc.vector.tensor_add(
                        cf[:, 2 * DC : 4 * DC],
                        cf[:, 2 * DC : 4 * DC],
                        cf[:, 6 * DC : 8 * DC],
                    )
                    nc.vector.tensor_add(
                        cf[:, 0 : 2 * DC], cf[:, 0 : 2 * DC], cf[:, 2 * DC : 4 * DC]
                    )
                    nc.vector.tensor_add(
                        cf[:, 0:DC], cf[:, 0:DC], cf[:, DC : 2 * DC]
                    )
                    for k in range(DC // 128):
                        kt = dc * (DC // 128) + k
                        ptr = gpsum.tile([128, 128], F32, tag="tp")
                        nc.tensor.transpose(
                            ptr[:],
                            chunk[:, 0, k * 128 : (k + 1) * 128],
                            ident_sb[:, :],
                        )
                        tst = gpool.tile([128, 128], F32, tag="tsT", bufs=4)
                        nc.vector.tensor_copy(tst[:], ptr[:])
                        nc.tensor.matmul(
                            gps[:],
                            lhsT=wg_sb[:, kt * E : (kt + 1) * E],
                            rhs=tst[:],
                            start=(kt == 0),
                            stop=False,
                        )
                    # release a W-f32 slot on the last two chunks
                    if ci >= NCH - 2:
                        di = ci - (NCH - 2)
                        scr = spool.tile([1, 1], F32, tag="scr")
                        nc.vector.tensor_add(
                            scr[:], wdum[di][:], chunk[0:1, 0, 0:1]
                        )
                    ci += 1
                nc.tensor.matmul(
                    gps[:],
                    lhsT=bg_sb[0:1, :],
                    rhs=ones_sb[0:1, 0:128],
                    start=False,
                    stop=True,
                )
                gT_sb = gpool.tile([E, 128], F32, tag="gT")
                nc.vector.tensor_copy(gT_sb[:], gps[:])
                gp = gpsumg.tile([128, E], F32, tag="tpg")
                nc.tensor.transpose(gp[:], gT_sb[:], ident_sb[0:E, 0:E])
                gate_sb = gpool.tile([128, E], F32, tag="gate")
                nc.vector.tensor_copy(gate_sb[:], gp[:])
                mxv = gpool.tile([128, 8], F32, tag="mxv")
                mxi = gpool.tile([128, 8], U32, tag="mxi")
                nc.vector.max_with_indices(mxv[:], mxi[:], gate_sb[:])
                nc.sync.dma_start(
                    top1_loc[tt * 128 : (tt + 1) * 128, :], mxi[:, 0:1]
                )

            # ---- W-f32 stream on the HWDGE queues + bf16 cast pipeline ---
            wfs = []
            wts = []

            def load_wf(e):
                wf = wfpool.tile([128, NKX * CS], F32, tag="wf", name=f"wf{e}")
                nc.sync.dma_start(
                    wf[:].rearrange("p (k n) -> p k n", k=NKX),
                    w_sh[e].rearrange("(k p) n -> p k n", p=128),
                )
                wfs.append(wf)

            def cast_w(e, eng):
                wt = wpool.tile([128, NKX * CS], BF16, tag="wt", name=f"wt{e}")
                eng.tensor_copy(wt[:], wfs[e][:])
                wts.append(wt)

            for e in range(8):
                load_wf(e)
            # first casts on DVE (safe: fewer than the wt pool depth)
            for e in range(5):
                cast_w(e, nc.vector)

            # ================= phase 2: one AllGather =====================
            # (sits after the gpsimd reduce ops -> triggers at gating end)
            nc.gpsimd.collective_compute(
                "AllGather",
                mybir.AluOpType.bypass,
                replica_groups=[list(range(NCORES))],
                ins=[top1_loc[:].opt()],
                outs=[all_top1[:].opt()],
            )
            nc.scalar.dma_start(top1_out[:, :], all_top1[:, :])

            # ================= phase 3: slot assignment (batched) =========
            tb_all = r1pool.tile([128, NT], U32)
            nc.scalar.dma_start(
                tb_all[:],
                all_top1[:, :].rearrange("(j p) one -> p (j one)", p=128),
            )
            t1f_all = r1pool.tile([128, NT], F32)
            nc.vector.tensor_copy(t1f_all[:], tb_all[:])
            oh_all = r1pool.tile([128, NT * E], F32)
            for i in range(NT):
                nc.vector.tensor_tensor(
                    out=oh_all[:, i * E : (i + 1) * E],
                    in0=t1f_all[:, i : i + 1].to_broadcast([128, E]),
                    in1=iota_e_sb[:],
                    op=mybir.AluOpType.is_equal,
                )
            # per-tile expert counts -> one psum row [1, NT*E]
            pcnt = gpsumg.tile([1, NT * E], F32, tag="tpg")
            nc.tensor.matmul(
                pcnt[:], lhsT=ones_sb[0:128, 0:1], rhs=oh_all[:],
                start=True, stop=True,
            )
            cnt_sb = r1pool.tile([1, NT * E], F32)
            nc.vector.tensor_copy(cnt_sb[:], pcnt[:])
            # counts2d [NT, E] via E strided mini-transposes
            pc2 = gpsumg.tile([NT, E], F32, tag="tpg")
            for e in range(E):
                nc.tensor.transpose(
                    pc2[:, e : e + 1],
                    cnt_sb[0:1, :].rearrange("one (i e) -> one i e", e=E)[:, :, e],
                    ident_sb[0:1, 0:1],
                )
            c2_sb = r1pool.tile([NT, E], F32)
            nc.vector.tensor_copy(c2_sb[:], pc2[:])
            # exclusive tile-base prefix: base2[i,e] = sum_{j<i} c2[j,e]
            pb2 = gpsumg.tile([NT, E], F32, tag="tpg")
            nc.tensor.matmul(
                pb2[:], lhsT=lsl_sb[0:NT, 0:NT], rhs=c2_sb[:],
                start=True, stop=True,
            )
            b2_sb = r1pool.tile([NT, E], F32)
            nc.vector.tensor_copy(b2_sb[:], pb2[:])

            # rank = within-tile exclusive prefix (one lsl matmul) + tile
            # base (per-tile partition-broadcast matmuls), separate psums
            pr1 = gpsumg.tile([128, NT * E], F32, tag="tpg")
            nc.tensor.matmul(
                pr1[:], lhsT=lsl_sb[:], rhs=oh_all[:],
                start=True, stop=True,
            )
            pr2 = gpsumg.tile([128, NT * E], F32, tag="tpg")
            for i in range(NT):
                nc.tensor.matmul(
                    pr2[:, i * E : (i + 1) * E],
                    lhsT=bcast16_sb[:, i * 128 : (i + 1) * 128],
                    rhs=b2_sb[:],
                    start=True,
                    stop=True,
                )
            sel = r1pool.tile([128, NT * E], F32)
            nc.vector.tensor_copy(sel[:], pr1[:])
            nc.vector.tensor_add(sel[:], sel[:], pr2[:])
            nc.vector.tensor_mul(sel[:], sel[:], oh_all[:])
            rank_all = r1pool.tile([128, NT], F32)
            for i in range(NT):
                nc.vector.reduce_sum(
                    rank_all[:, i : i + 1],
                    sel[:, i * E : (i + 1) * E],
                    axis=mybir.AxisListType.X,
                )
            # table row r = (rank%128)*NA + top1*NMT + (rank>=128):
            # partition-major layout makes every table DMA contiguous
            ge = r1pool.tile([128, NT], F32)
            nc.vector.tensor_scalar(
                ge[:], rank_all[:], 128.0, scalar2=None, op0=mybir.AluOpType.is_ge
            )
            rem = r1pool.tile([128, NT], F32)
            nc.vector.tensor_scalar(
                rem[:], ge[:], 128.0, scalar2=None, op0=mybir.AluOpType.mult
            )
            nc.vector.tensor_tensor(
                out=rem[:], in0=rank_all[:], in1=rem[:],
                op=mybir.AluOpType.subtract,
            )
            posf = r1pool.tile([128, NT], F32)
            nc.vector.tensor_scalar(
                posf[:], rem[:], float(NA), scalar2=None, op0=mybir.AluOpType.mult
            )
            t2 = r1pool.tile([128, NT], F32)
            nc.vector.tensor_scalar(
                t2[:], t1f_all[:], float(NMT), scalar2=None,
                op0=mybir.AluOpType.mult,
            )
            nc.vector.tensor_add(posf[:], posf[:], t2[:])
            nc.vector.tensor_add(posf[:], posf[:], ge[:])
            posu = r1pool.tile([128, NT], U32)
            nc.vector.tensor_copy(posu[:], posf[:])
            # 16 independent scatters (no WAW -> pipeline at emission rate)
            for j in range(NT):
                nc.gpsimd.indirect_dma_start(
                    out=tabs[j][:, :],
                    out_offset=IndirectOffsetOnAxis(ap=posu[:, j : j + 1], axis=0),
                    in_=tokid4_sb[:, j * ROWW : (j + 1) * ROWW],
                    in_offset=None,
                    bounds_check=SLOTS - 1,
                    oob_is_err=False,
                )
            # contiguous read-back + min-merge into the slot->token map
            pslice = r1pool.tile([128, NA * ROWW], U32)
            for j in range(NT):
                m = mpool.tile([128, NA * ROWW], U32, tag="mg")
                nc.scalar.dma_start(
                    m[:],
                    tabs[j][:, :].rearrange("(p q) n -> p (q n)", p=128),
                )
                if j == 0:
                    nc.vector.tensor_copy(pslice[:], m[:])
                else:
                    nc.vector.tensor_tensor(
                        out=pslice[:], in0=pslice[:], in1=m[:],
                        op=mybir.AluOpType.min,
                    )

            # ================= phase 4: expert matmul =====================
            def gather_x(e, mt):
                a = e * NMT + mt
                xg = xpool.tile([128, IN], F32R, tag="xg")
                nc.gpsimd.indirect_dma_start(
                    out=xg[:],
                    out_offset=None,
                    in_=x_full[:, :],
                    in_offset=IndirectOffsetOnAxis(
                        ap=pslice[:, a * ROWW : a * ROWW + 1], axis=0
                    ),
                    bounds_check=B - 1,
                    oob_is_err=False,
                )
                return xg

            cast_w(5, nc.gpsimd)
            xgs = {}
            for e in range(2):
                for mt in range(NMT):
                    xgs[(e, mt)] = gather_x(e, mt)

            for e in range(E):
                wt = wts[e]
                for mt in range(NMT):
                    xg = xgs.pop((e, mt))
                    xgT = xpool.tile([128, IN], BF16, tag="xgT")
                    for k in range(NKX):
                        ptx = gpsum.tile([128, 128], F32R, tag="tp")
                        nc.tensor.transpose(
                            ptx[:],
                            xg[:, k * 128 : (k + 1) * 128],
                            identr_sb[:, :],
                        )
                        nc.vector.tensor_copy(
                            xgT[:, k * 128 : (k + 1) * 128], ptx[:]
                        )
                    po = opsum.tile([128, CS], F32, tag="po")
                    for k in range(NKX):
                        nc.tensor.matmul(
                            po[:],
                            lhsT=xgT[:, k * 128 : (k + 1) * 128],
                            rhs=wt[:, k * CS : (k + 1) * CS],
                            start=(k == 0),
                            stop=False,
                        )
                    nc.tensor.matmul(
                        po[:],
                        lhsT=ones_bf[0:1, :],
                        rhs=bias_bf[0:1, e * CS : (e + 1) * CS],
                        start=False,
                        stop=True,
                    )
                    ot = opool.tile([128, CS], BF16, tag="ot")
                    nc.vector.tensor_copy(ot[:], po[:])
                    nc.sync.dma_start(
                        out_slots[(e * NMT + mt) * 128 : (e * NMT + mt + 1) * 128, :],
                        ot[:],
                    )
                # software pipeline: next gathers, W casts, W-f32 loads
                if e + 2 < E:
                    for mt in range(NMT):
                        xgs[(e + 2, mt)] = gather_x(e + 2, mt)
                if e + 6 < E:
                    cast_w(e + 6, nc.gpsimd)
                if e + 8 < E:
                    load_wf(e + 8)

    nc.compile()
    return nc


def make_in_maps(inputs: dict) -> list[dict]:
    x = np.ascontiguousarray(np.asarray(inputs["x"], dtype=np.float32))
    t = np.ascontiguousarray(np.asarray(inputs["t"], dtype=np.float32))
    W = np.ascontiguousarray(np.asarray(inputs["W"], dtype=np.float32))
    b = np.ascontiguousarray(np.asarray(inputs["b"], dtype=np.float32))
    Wg = np.ascontiguousarray(np.asarray(inputs["Wg"], dtype=np.float32))
    bg = np.ascontiguousarray(np.asarray(inputs["bg"], dtype=np.float32))

    x2 = np.ascontiguousarray(x[:, 0, :])                       # [B, IN]
    ident = np.eye(128, dtype=np.float32)
    lsl = np.triu(np.ones((128, 128), np.float32), k=1)          # lsl[r,c]=1 iff r<c
    # bcast16[j, i*128+p] = 1 iff i == j  (base-row broadcast selector)
    bcast16 = np.zeros((NT, NT * 128), np.float32)
    for i in range(NT):
        bcast16[i, i * 128 : (i + 1) * 128] = 1.0
    iota_e = np.tile(np.arange(E, dtype=np.float32)[None, :], (128, 1))
    # tokid4[p, j*ROWW+r] = global token id of (tile j, row p); tile j holds
    # tokens g = (j//NTT)*BS + (j%NTT)*128 + p (single AG -> global order)
    tokid4 = np.zeros((128, NT * ROWW), np.uint32)
    for j in range(NT):
        tokid4[:, j * ROWW : (j + 1) * ROWW] = (
            j * 128 + np.arange(128, dtype=np.uint32)[:, None]
        )

    in_maps = []
    for c in range(NCORES):
        cs = slice(c * CS, (c + 1) * CS)
        in_maps.append({
            "t_sh": np.ascontiguousarray(t[c * BS : (c + 1) * BS]),
            "x_full": x2,
            "w_sh": np.ascontiguousarray(W[:, :, cs]),
            "b_sh": np.ascontiguousarray(b[:, cs]).reshape(1, E * CS),
            "wg_s": np.ascontiguousarray(Wg / float(T)),
            "bg_r": bg.reshape(1, E),
            "ident": ident,
            "identr": ident,
            "lsl": lsl,
            "bcast16": bcast16,
            "iota_e": iota_e,
            "tokid4": tokid4,
        })
    return in_maps


def assemble_output(per_core_results: list[dict]) -> np.ndarray:
    top1 = np.asarray(per_core_results[0]["top1_out"]).reshape(B).astype(np.int64)
    # recompute slot(token) exactly as the device did (stable within-expert
    # rank over global token order; single AG -> tiles are g//128)
    rank = np.zeros(B, dtype=np.int64)
    counts = np.zeros(E, dtype=np.int64)
    for g in range(B):
        e = top1[g]
        rank[g] = counts[e]
        counts[e] += 1
    assert counts.max() <= CAP, f"expert overflow: {counts}"
    slot = top1 * CAP + rank
    out = np.empty((B, 1, OUT), dtype=np.float32)
    for c in range(NCORES):
        osl = np.asarray(per_core_results[c]["out_slots"]).astype(np.float32)
        out[:, 0, c * CS : (c + 1) * CS] = osl[slot]
    return out


_NC_CACHE = {}


def kernel(**inputs) -> np.ndarray:
    if "nc" not in _NC_CACHE:
        _NC_CACHE["nc"] = build_kernel()
    nc = _NC_CACHE["nc"]
    in_maps = make_in_maps(inputs)
    res = run_bass_kernel_spmd(nc, in_maps, core_ids=list(range(NCORES)))
    return assemble_output(res.results)


# revision 27
# speedup vs baseline: 1.2850x; 1.0273x over previous
"""Top-1 MoE mapper kernel for Trainium2, SPMD over 8 NeuronCores.

Problem (hardcoded shapes):
  x  [2048, 1, 1024] f32   token inputs
  t  [2048, 8, 4096] f32   gating context
  W  [12, 1024, 4096] f32  expert weights
  b  [12, 4096] f32        expert biases
  Wg [4096, 12] f32        gate weights
  bg [12] f32              gate bias
  out[b] = x[b] @ W[argmax(t[b].mean(T) @ Wg + bg)] + b[...]  -> [2048, 1, 4096]

Strategy (v6):
  - Gating data-parallel over B: 8 x 4MB t-chunks alternate across the two
    HWDGE queues (4KB descriptors; 2MB chunks only reached ~210GB/s). The
    T-tree-reduce level 1 is split across GpSimd and DVE so neither engine
    paces the stream; PSUM->SBUF copies stay on DVE (ACT is 9x slower).
    Gating is f32 end-to-end so the device top-1 matches the reference.
  - ONE AllGather (gpsimd-triggered, sitting right after the gpsimd reduce
    ops so it triggers at gating end; the collective's ~40us arm latency is
    unavoidable, so the W stream fills that window).
  - W streams as f32 HALF-EXPERT loads on the otherwise-idle HWDGE queues;
    the first two are held back by dummy-slot WARs released by the last
    gating chunks. All f32->bf16 W casts run on DVE into a 7-slot bf16 pool
    (deep enough that in-loop casts never stall the DVE FIFO).
  - Routing batched: one-hots; replicated per-tile counts via ONE matmul;
    tile-base prefix via mask-multiply + reduce (no transpose chain);
    within-tile rank via one lsl matmul + per-tile base broadcasts. The
    slot->token scatter goes to 16 INDEPENDENT sentinel-initialized
    sub-tables stored PARTITION-MAJOR (row r = p*24 + a for slot a*128+p,
    computed on DVE) so init and read-back are contiguous DMAs and the 16
    scatters pipeline at emission rate; tables min-merge on DVE into the
    SBUF slot->token map. Padded slots keep the sentinel and drop their
    gather traffic (bounds_check, oob_is_err=False).
  - Expert matmul output-column-parallel (core c owns 512 output columns):
    per m-tile a 128-row indirect gather of x (the only gpsimd-queue work in
    phase 4), PE transposes, bf16 matmuls (N=512), bf16 bias via K=1 matmul,
    bf16 results written contiguously in slot order.
  - Host unpermutes slots -> tokens using the device-computed top-1 ids
    (pure data movement; all routing math happens on device).
"""

import numpy as np

import concourse.bass as bass
import concourse.bacc as bacc
import concourse.mybir as mybir
import concourse.tile as tile
from concourse.bass import IndirectOffsetOnAxis
from concourse.bass_utils import run_bass_kernel_spmd

F32 = mybir.dt.float32
F32R = mybir.dt.float32r
BF16 = mybir.dt.bfloat16
U32 = mybir.dt.uint32

B, T, IN, OUT, E = 2048, 8, 1024, 4096, 12
NCORES = 8
BS = B // NCORES            # 256 tokens per core (gating shard)
CS = OUT // NCORES          # 512 output columns per core (expert shard)
CAP = 256                   # capacity slots per expert
SLOTS = E * CAP             # 3072
NT = B // 128               # 16 token tiles globally
NTT = BS // 128             # 2 token tiles per core
NKX = IN // 128             # 8 k-tiles over the expert contraction
NMT = CAP // 128            # 2 m-tiles per expert
DC = 1024                   # gating d-chunk width
NDC = OUT // DC             # 4 chunks per token tile
NCH = NTT * NDC             # 8 chunks total
ROWW = 4                    # u32 per slot-table row (16B rows)
NA = SLOTS // 128           # 24 slot blocks (m-tiles across all experts)
NWH = 2 * E                 # 24 half-expert W loads
SENTINEL = 3000000000.0     # > B-1 as u32 -> min-merge keeps real ids


def build_kernel(enable_asserts: bool = False):
    nc = bacc.Bacc(
        "TRN2",
        target_bir_lowering=False,
        debug=False,
        enable_asserts=enable_asserts,
        num_devices=NCORES,
    )

    # ---- I/O -------------------------------------------------------------
    t_sh = nc.dram_tensor("t_sh", [BS, T, OUT], F32, kind="ExternalInput")
    x_full = nc.dram_tensor("x_full", [B, IN], F32R, kind="ExternalInput")
    w_sh = nc.dram_tensor("w_sh", [E, IN, CS], F32, kind="ExternalInput")
    b_sh = nc.dram_tensor("b_sh", [1, E * CS], F32, kind="ExternalInput")
    wg_s = nc.dram_tensor("wg_s", [OUT, E], F32, kind="ExternalInput")  # Wg/T
    bg_r = nc.dram_tensor("bg_r", [1, E], F32, kind="ExternalInput")
    ident = nc.dram_tensor("ident", [128, 128], F32, kind="ExternalInput")
    identr = nc.dram_tensor("identr", [128, 128], F32R, kind="ExternalInput")
    lsl = nc.dram_tensor("lsl", [128, 128], F32, kind="ExternalInput")
    bcast16 = nc.dram_tensor("bcast16", [NT, NT * 128], F32, kind="ExternalInput")
    ltmask16 = nc.dram_tensor("ltmask16", [NT, NT * E], F32, kind="ExternalInput")
    iota_e = nc.dram_tensor("iota_e", [128, E], F32, kind="ExternalInput")
    tokid4 = nc.dram_tensor("tokid4", [128, NT * ROWW], U32, kind="ExternalInput")

    out_slots = nc.dram_tensor("out_slots", [SLOTS, CS], BF16, kind="ExternalOutput")
    top1_out = nc.dram_tensor("top1_out", [B, 1], U32, kind="ExternalOutput")

    with tile.TileContext(nc) as tc:
        with (
            tc.tile_pool(name="consts", bufs=1) as cpool,
            tc.tile_pool(name="dram", bufs=1, space="DRAM") as dpool,
            tc.tile_pool(name="wf", bufs=2) as wfpool,
            tc.tile_pool(name="wp", bufs=7) as wpool,
            tc.tile_pool(name="gat", bufs=2) as gpool,
            tc.tile_pool(name="gat1", bufs=1) as g1pool,
            # PSUM budget (8 banks): tp x2 + tpg x2 + gps x2 + po x2
            tc.tile_pool(name="gps", bufs=2, space="PSUM") as gpsum,
            tc.tile_pool(name="gpsg", bufs=2, space="PSUM") as gpsumg,
            tc.tile_pool(name="gps1", bufs=2, space="PSUM") as gpsum1,
            tc.tile_pool(name="rout", bufs=1) as r1pool,
            tc.tile_pool(name="mrg", bufs=3) as mpool,
            tc.tile_pool(name="scr", bufs=4) as spool,
            tc.tile_pool(name="xp", bufs=4) as xpool,
            tc.tile_pool(name="op", bufs=3) as opool,
            tc.tile_pool(name="ops", bufs=2, space="PSUM") as opsum,
        ):
            # ---- dummies holding the first two W-f32 slots until the last
            # gating chunks release them (keeps phase-1 HBM for t) ---------
            wdum = []
            for i in range(2):
                dm = wfpool.tile([1, 1], F32, tag="wf", name=f"wdum{i}")
                nc.vector.memset(dm[:], 0.0)
                wdum.append(dm)

            # ---- constants resident in SBUF for the whole kernel ---------
            ident_sb = cpool.tile([128, 128], F32)
            nc.scalar.dma_start(ident_sb[:], ident[:, :])
            identr_sb = cpool.tile([128, 128], F32R)
            nc.scalar.dma_start(identr_sb[:], identr[:, :])
            lsl_sb = cpool.tile([128, 128], F32)
            nc.scalar.dma_start(lsl_sb[:], lsl[:, :])
            bcast16_sb = cpool.tile([NT, NT * 128], F32)
            nc.scalar.dma_start(bcast16_sb[:], bcast16[:, :])
            ltmask16_sb = cpool.tile([NT, NT * E], F32)
            nc.scalar.dma_start(ltmask16_sb[:], ltmask16[:, :])
            iota_e_sb = cpool.tile([128, E], F32)
            nc.scalar.dma_start(iota_e_sb[:], iota_e[:, :])
            tokid4_sb = cpool.tile([128, NT * ROWW], U32)
            nc.scalar.dma_start(tokid4_sb[:], tokid4[:, :])
            ones_sb = cpool.tile([128, 128], F32)
            nc.vector.memset(ones_sb[:], 1.0)
            ones_bf = cpool.tile([1, 128], BF16)
            nc.vector.memset(ones_bf[:], 1.0)
            # Wg/T laid out [128, 32*E]: wg_sb[p, kt*E+e] = Wg[kt*128+p, e]
            wg_sb = cpool.tile([128, (OUT // 128) * E], F32)
            nc.scalar.dma_start(
                wg_sb[:].rearrange("p (k e) -> p k e", e=E),
                wg_s[:, :].rearrange("(k p) e -> p k e", p=128),
            )
            bg_sb = cpool.tile([1, E], F32)
            nc.scalar.dma_start(bg_sb[:], bg_r[:, :])
            bias_bf = cpool.tile([1, E * CS], BF16)
            nc.gpsimd.dma_start(bias_bf[:], b_sh[:, :])
            sent_sb = cpool.tile([128, NA * ROWW], U32)
            nc.vector.memset(sent_sb[:], SENTINEL)

            # DRAM scratch
            top1_loc = dpool.tile([BS, 1], U32, name="t1loc")
            all_top1 = dpool.tile([B, 1], U32, name="allt1")
            # 16 independent slot->token sub-tables, PARTITION-MAJOR rows:
            # row r = p*NA + a holds the token of slot a*128+p
            tabs = [
                dpool.tile([SLOTS, ROWW], U32, name=f"tab{j}") for j in range(NT)
            ]

            # ================= phase 1: gating ============================
            qeng = [nc.scalar, nc.sync]
            ci = 0
            for tt in range(NTT):
                gps = gpsum1.tile([E, 128], F32, tag="gps")
                for dc in range(NDC):
                    chunk = gpool.tile([128, T, DC], F32, tag="tchunk")
                    qeng[ci % 2].dma_start(
                        chunk[:],
                        t_sh[tt * 128 : (tt + 1) * 128, :, dc * DC : (dc + 1) * DC],
                    )
                    # tree-reduce over T=8 into chunk[:, 0, :] -- exact f32.
                    # Level 1 split across GpSimd and DVE.
                    cf = chunk[:].rearrange("p t d -> p (t d)")
                    nc.gpsimd.tensor_add(
                        cf[:, 0 : 2 * DC], cf[:, 0 : 2 * DC], cf[:, 4 * DC : 6 * DC]
                    )
                    nc.vector.tensor_add(
                        cf[:, 2 * DC : 4 * DC],
                        cf[:, 2 * DC : 4 * DC],
                        cf[:, 6 * DC : 8 * DC],
                    )
                    nc.vector.tensor_add(
                        cf[:, 0 : 2 * DC], cf[:, 0 : 2 * DC], cf[:, 2 * DC : 4 * DC]
                    )
                    nc.vector.tensor_add(
                        cf[:, 0:DC], cf[:, 0:DC], cf[:, DC : 2 * DC]
                    )
                    for k in range(DC // 128):
                        kt = dc * (DC // 128) + k
                        ptr = gpsum.tile([128, 128], F32, tag="tp")
                        nc.tensor.transpose(
                            ptr[:],
                            chunk[:, 0, k * 128 : (k + 1) * 128],
                            ident_sb[:, :],
                        )
                        tst = gpool.tile([128, 128], F32, tag="tsT", bufs=4)
                        nc.vector.tensor_copy(tst[:], ptr[:])
                        nc.tensor.matmul(
                            gps[:],
                            lhsT=wg_sb[:, kt * E : (kt + 1) * E],
                            rhs=tst[:],
                            start=(kt == 0),
                            stop=False,
                        )
                    # release a W-f32 slot on the last two chunks
                    if ci >= NCH - 2:
                        di = ci - (NCH - 2)
                        scr = spool.tile([1, 1], F32, tag="scr")
                        nc.vector.tensor_add(
                            scr[:], wdum[di][:], chunk[0:1, 0, 0:1]
                        )
                    ci += 1
                nc.tensor.matmul(
                    gps[:],
                    lhsT=bg_sb[0:1, :],
                    rhs=ones_sb[0:1, 0:128],
                    start=False,
                    stop=True,
                )
                gT_sb = gpool.tile([E, 128], F32, tag="gT")
                nc.vector.tensor_copy(gT_sb[:], gps[:])
                gp = gpsumg.tile([128, E], F32, tag="tpg")
                nc.tensor.transpose(gp[:], gT_sb[:], ident_sb[0:E, 0:E])
                gate_sb = gpool.tile([128, E], F32, tag="gate")
                nc.vector.tensor_copy(gate_sb[:], gp[:])
                mxv = gpool.tile([128, 8], F32, tag="mxv")
                mxi = gpool.tile([128, 8], U32, tag="mxi")
                nc.vector.max_with_indices(mxv[:], mxi[:], gate_sb[:])
                nc.sync.dma_start(
                    top1_loc[tt * 128 : (tt + 1) * 128, :], mxi[:, 0:1]
                )

            # sentinel-init the sub-tables (contiguous; needed by scatter
            # time only, so they sit behind the gating chunks in the FIFO)
            for j in range(NT):
                nc.scalar.dma_start(
                    tabs[j][:, :].rearrange("(p q) n -> p (q n)", p=128),
                    sent_sb[:],
                )

            # ---- W-f32 half-expert stream + DVE bf16 cast pipeline -------
            wfhs = []
            wts = []

            def load_wfh(k):
                wf = wfpool.tile([128, (NKX // 2) * CS], F32, tag="wf",
                                 name=f"wf{k}")
                nc.sync.dma_start(
                    wf[:].rearrange("p (k n) -> p k n", k=NKX // 2),
                    w_sh[k // 2].rearrange("(k p) n -> p k n", p=128)[
                        :, (k % 2) * (NKX // 2) : (k % 2 + 1) * (NKX // 2), :
                    ],
                )
                wfhs.append(wf)

            def cast_wh(k):
                e, h = k // 2, k % 2
                if h == 0:
                    wt = wpool.tile([128, NKX * CS], BF16, tag="wt",
                                    name=f"wt{e}")
                    wts.append(wt)
                half = (NKX // 2) * CS
                nc.vector.tensor_copy(
                    wts[e][:, h * half : (h + 1) * half], wfhs[k][:]
                )

            for k in range(12):
                load_wfh(k)
            for k in range(10):
                cast_wh(k)

            # ================= phase 2: one AllGather =====================
            # (sits after the gpsimd reduce ops -> triggers at gating end)
            nc.gpsimd.collective_compute(
                "AllGather",
                mybir.AluOpType.bypass,
                replica_groups=[list(range(NCORES))],
                ins=[top1_loc[:].opt()],
                outs=[all_top1[:].opt()],
            )
            nc.scalar.dma_start(top1_out[:, :], all_top1[:, :])

            # ================= phase 3: slot assignment (batched) =========
            tb_all = r1pool.tile([128, NT], U32)
            nc.scalar.dma_start(
                tb_all[:],
                all_top1[:, :].rearrange("(j p) one -> p (j one)", p=128),
            )
            t1f_all = r1pool.tile([128, NT], F32)
            nc.vector.tensor_copy(t1f_all[:], tb_all[:])
            oh_all = r1pool.tile([128, NT * E], F32)
            for i in range(NT):
                nc.vector.tensor_tensor(
                    out=oh_all[:, i * E : (i + 1) * E],
                    in0=t1f_all[:, i : i + 1].to_broadcast([128, E]),
                    in1=iota_e_sb[:],
                    op=mybir.AluOpType.is_equal,
                )
            # replicated per-tile counts in one matmul: crep[j, j'E+e] =
            # cnt[j'][e] for every j; tile-base prefix via mask + reduce
            crep = gpsumg.tile([NT, NT * E], F32, tag="tpg")
            nc.tensor.matmul(
                crep[:], lhsT=ones_sb[:, 0:NT], rhs=oh_all[:],
                start=True, stop=True,
            )
            cmask = r1pool.tile([NT, NT * E], F32)
            nc.vector.tensor_mul(cmask[:], crep[:], ltmask16_sb[:])
            b2_sb = r1pool.tile([NT, E], F32)
            nc.vector.reduce_sum(
                b2_sb[:],
                cmask[:].rearrange("p (j e) -> p e j", e=E),
                axis=mybir.AxisListType.X,
            )

            # rank = within-tile exclusive prefix (one lsl matmul) + tile
            # base (per-tile partition-broadcast matmuls), separate psums
            pr1 = gpsumg.tile([128, NT * E], F32, tag="tpg")
            nc.tensor.matmul(
                pr1[:], lhsT=lsl_sb[:], rhs=oh_all[:],
                start=True, stop=True,
            )
            pr2 = gpsumg.tile([128, NT * E], F32, tag="tpg")
            for i in range(NT):
                nc.tensor.matmul(
                    pr2[:, i * E : (i + 1) * E],
                    lhsT=bcast16_sb[:, i * 128 : (i + 1) * 128],
                    rhs=b2_sb[:],
                    start=True,
                    stop=True,
                )
            sel = r1pool.tile([128, NT * E], F32)
            nc.vector.tensor_copy(sel[:], pr1[:])
            nc.vector.tensor_add(sel[:], sel[:], pr2[:])
            nc.vector.tensor_mul(sel[:], sel[:], oh_all[:])
            rank_all = r1pool.tile([128, NT], F32)
            for i in range(NT):
                nc.vector.reduce_sum(
                    rank_all[:, i : i + 1],
                    sel[:, i * E : (i + 1) * E],
                    axis=mybir.AxisListType.X,
                )
            # table row r = (rank%128)*NA + top1*NMT + (rank>=128):
            # partition-major layout makes every table DMA contiguous
            ge = r1pool.tile([128, NT], F32)
            nc.vector.tensor_scalar(
                ge[:], rank_all[:], 128.0, scalar2=None, op0=mybir.AluOpType.is_ge
            )
            rem = r1pool.tile([128, NT], F32)
            nc.vector.tensor_scalar(
                rem[:], ge[:], 128.0, scalar2=None, op0=mybir.AluOpType.mult
            )
            nc.vector.tensor_tensor(
                out=rem[:], in0=rank_all[:], in1=rem[:],
                op=mybir.AluOpType.subtract,
            )
            posf = r1pool.tile([128, NT], F32)
            nc.vector.tensor_scalar(
                posf[:], rem[:], float(NA), scalar2=None, op0=mybir.AluOpType.mult
            )
            t2 = r1pool.tile([128, NT], F32)
            nc.vector.tensor_scalar(
                t2[:], t1f_all[:], float(NMT), scalar2=None,
                op0=mybir.AluOpType.mult,
            )
            nc.vector.tensor_add(posf[:], posf[:], t2[:])
            nc.vector.tensor_add(posf[:], posf[:], ge[:])
            posu = r1pool.tile([128, NT], U32)
            nc.vector.tensor_copy(posu[:], posf[:])
            # 16 independent scatters (no WAW -> pipeline at emission rate)
            for j in range(NT):
                nc.gpsimd.indirect_dma_start(
                    out=tabs[j][:, :],
                    out_offset=IndirectOffsetOnAxis(ap=posu[:, j : j + 1], axis=0),
                    in_=tokid4_sb[:, j * ROWW : (j + 1) * ROWW],
                    in_offset=None,
                    bounds_check=SLOTS - 1,
                    oob_is_err=False,
                )
            # contiguous read-back + min-merge into the slot->token map
            pslice = r1pool.tile([128, NA * ROWW], U32)
            for j in range(NT):
                m = mpool.tile([128, NA * ROWW], U32, tag="mg")
                nc.scalar.dma_start(
                    m[:],
                    tabs[j][:, :].rearrange("(p q) n -> p (q n)", p=128),
                )
                if j == 0:
                    nc.vector.tensor_copy(pslice[:], m[:])
                else:
                    nc.vector.tensor_tensor(
                        out=pslice[:], in0=pslice[:], in1=m[:],
                        op=mybir.AluOpType.min,
                    )

            # ================= phase 4: expert matmul =====================
            def gather_x(e, mt):
                a = e * NMT + mt
                xg = xpool.tile([128, IN], F32R, tag="xg")
                nc.gpsimd.indirect_dma_start(
                    out=xg[:],
                    out_offset=None,
                    in_=x_full[:, :],
                    in_offset=IndirectOffsetOnAxis(
                        ap=pslice[:, a * ROWW : a * ROWW + 1], axis=0
                    ),
                    bounds_check=B - 1,
                    oob_is_err=False,
                )
                return xg

            xgs = {}
            for e in range(2):
                for mt in range(NMT):
                    xgs[(e, mt)] = gather_x(e, mt)

            for e in range(E):
                wt = wts[e]
                for mt in range(NMT):
                    xg = xgs.pop((e, mt))
                    xgT = xpool.tile([128, IN], BF16, tag="xgT")
                    for k in range(NKX):
                        ptx = gpsum.tile([128, 128], F32R, tag="tp")
                        nc.tensor.transpose(
                            ptx[:],
                            xg[:, k * 128 : (k + 1) * 128],
                            identr_sb[:, :],
                        )
                        nc.vector.tensor_copy(
                            xgT[:, k * 128 : (k + 1) * 128], ptx[:]
                        )
                    po = opsum.tile([128, CS], F32, tag="po")
                    for k in range(NKX):
                        nc.tensor.matmul(
                            po[:],
                            lhsT=xgT[:, k * 128 : (k + 1) * 128],
                            rhs=wt[:, k * CS : (k + 1) * CS],
                            start=(k == 0),
                            stop=False,
                        )
                    nc.tensor.matmul(
                        po[:],
                        lhsT=ones_bf[0:1, :],
                        rhs=bias_bf[0:1, e * CS : (e + 1) * CS],
                        start=False,
                        stop=True,
                    )
                    ot = opool.tile([128, CS], BF16, tag="ot")
                    nc.vector.tensor_copy(ot[:], po[:])
                    nc.sync.dma_start(
                        out_slots[(e * NMT + mt) * 128 : (e * NMT + mt + 1) * 128, :],
                        ot[:],
                    )
                # software pipeline: next gathers, W casts, W-f32 loads
                if e + 2 < E:
                    for mt in range(NMT):
                        xgs[(e + 2, mt)] = gather_x(e + 2, mt)
                for k in (2 * e + 10, 2 * e + 11):
                    if k < NWH:
                        cast_wh(k)
                for k in (2 * e + 12, 2 * e + 13):
                    if k < NWH:
                        load_wfh(k)

    nc.compile()
    return nc


def make_in_maps(inputs: dict) -> list[dict]:
    x = np.ascontiguousarray(np.asarray(inputs["x"], dtype=np.float32))
    t = np.ascontiguousarray(np.asarray(inputs["t"], dtype=np.float32))
    W = np.ascontiguousarray(np.asarray(inputs["W"], dtype=np.float32))
    b = np.ascontiguousarray(np.asarray(inputs["b"], dtype=np.float32))
    Wg = np.ascontiguousarray(np.asarray(inputs["Wg"], dtype=np.float32))
    bg = np.ascontiguousarray(np.asarray(inputs["bg"], dtype=np.float32))

    x2 = np.ascontiguousarray(x[:, 0, :])                       # [B, IN]
    ident = np.eye(128, dtype=np.float32)
    lsl = np.triu(np.ones((128, 128), np.float32), k=1)          # lsl[r,c]=1 iff r<c
    # bcast16[j, i*128+p] = 1 iff i == j  (base-row broadcast selector)
    bcast16 = np.zeros((NT, NT * 128), np.float32)
    for i in range(NT):
        bcast16[i, i * 128 : (i + 1) * 128] = 1.0
    # ltmask16[j, j'*E+e] = 1 iff j' < j  (tile-base prefix mask)
    ltmask16 = np.zeros((NT, NT * E), np.float32)
    for j in range(NT):
        for jp in range(j):
            ltmask16[j, jp * E : (jp + 1) * E] = 1.0
    iota_e = np.tile(np.arange(E, dtype=np.float32)[None, :], (128, 1))
    # tokid4[p, j*ROWW+r] = global token id j*128+p
    tokid4 = np.zeros((128, NT * ROWW), np.uint32)
    for j in range(NT):
        tokid4[:, j * ROWW : (j + 1) * ROWW] = (
            j * 128 + np.arange(128, dtype=np.uint32)[:, None]
        )

    in_maps = []
    for c in range(NCORES):
        cs = slice(c * CS, (c + 1) * CS)
        in_maps.append({
            "t_sh": np.ascontiguousarray(t[c * BS : (c + 1) * BS]),
            "x_full": x2,
            "w_sh": np.ascontiguousarray(W[:, :, cs]),
            "b_sh": np.ascontiguousarray(b[:, cs]).reshape(1, E * CS),
            "wg_s": np.ascontiguousarray(Wg / float(T)),
            "bg_r": bg.reshape(1, E),
            "ident": ident,
            "identr": ident,
            "lsl": lsl,
            "bcast16": bcast16,
            "ltmask16": ltmask16,
            "iota_e": iota_e,
            "tokid4": tokid4,
        })
    return in_maps


def assemble_output(per_core_results: list[dict]) -> np.ndarray:
    top1 = np.asarray(per_core_results[0]["top1_out"]).reshape(B).astype(np.int64)
    # recompute slot(token) exactly as the device did (stable within-expert
    # rank over global token order)
    rank = np.zeros(B, dtype=np.int64)
    counts = np.zeros(E, dtype=np.int64)
    for g in range(B):
        e = top1[g]
        rank[g] = counts[e]
        counts[e] += 1
    assert counts.max() <= CAP, f"expert overflow: {counts}"
    slot = top1 * CAP + rank
    out = np.empty((B, 1, OUT), dtype=np.float32)
    for c in range(NCORES):
        osl = np.asarray(per_core_results[c]["out_slots"]).astype(np.float32)
        out[:, 0, c * CS : (c + 1) * CS] = osl[slot]
    return out


_NC_CACHE = {}


def kernel(**inputs) -> np.ndarray:
    if "nc" not in _NC_CACHE:
        _NC_CACHE["nc"] = build_kernel()
    nc = _NC_CACHE["nc"]
    in_maps = make_in_maps(inputs)
    res = run_bass_kernel_spmd(nc, in_maps, core_ids=list(range(NCORES)))
    return assemble_output(res.results)


# revision 29
# speedup vs baseline: 1.3804x; 1.0742x over previous
"""Top-1 MoE mapper kernel for Trainium2, SPMD over 8 NeuronCores.

Problem (hardcoded shapes):
  x  [2048, 1, 1024] f32   token inputs
  t  [2048, 8, 4096] f32   gating context
  W  [12, 1024, 4096] f32  expert weights
  b  [12, 4096] f32        expert biases
  Wg [4096, 12] f32        gate weights
  bg [12] f32              gate bias
  out[b] = x[b] @ W[argmax(t[b].mean(T) @ Wg + bg)] + b[...]  -> [2048, 1, 4096]

Strategy (v7):
  - Gating data-parallel over B. Each 4MB t-chunk is fetched as TWO 2MB
    halves: t[:, 0:4, :] as a plain HWDGE load and t[:, 4:8, :] as a SWDGE
    ACCUMULATE-DMA onto the same SBUF tile -- the first reduction level
    happens inside the DMA engines (CCE f32 add), halving the SBUF chunk
    footprint and leaving DVE only ~3.3us of adds per chunk. The remaining
    tree levels, PE transposes, f32 gate matmul and argmax are unchanged.
    Gating is f32 end-to-end so the device top-1 matches the reference.
  - A zero-cost WARM-UP AllGather at the top of the gpsimd FIFO acts as a
    cross-core barrier and pre-arms the collective firmware while phase 1
    streams; the REAL AllGather (after the reduce ops) then pays less of
    the ~40us arm/skew latency.
  - W streams as f32 HALF-EXPERT loads on the HWDGE queues, first two held
    back by dummy-slot WARs released by the last gating chunks; all
    f32->bf16 W casts on DVE into a 7-slot bf16 pool.
  - Routing fully batched (single-op one-hots / counts matmul / mask-reduce
    tile bases / lsl rank matmul / single-op rank reduce). The slot->token
    scatter goes to 16 INDEPENDENT sentinel-initialized sub-tables stored
    PARTITION-MAJOR (row r = p*24 + a for slot a*128+p) so init and
    read-back are contiguous DMAs and the 16 scatters pipeline at emission
    rate; tables min-merge on DVE into the SBUF slot->token map. Padded
    slots keep the sentinel and drop their gather traffic.
  - Expert matmul output-column-parallel: per m-tile a 128-row indirect
    gather of x (the only gpsimd-queue work in phase 4), PE transposes,
    bf16 matmuls (N=512), bf16 bias via K=1 matmul, bf16 results written
    contiguously in slot order.
  - Host unpermutes slots -> tokens using the device-computed top-1 ids
    (pure data movement; all routing math happens on device).
"""

import numpy as np

import concourse.bass as bass
import concourse.bacc as bacc
import concourse.mybir as mybir
import concourse.tile as tile
from concourse.bass import IndirectOffsetOnAxis
from concourse.bass_utils import run_bass_kernel_spmd

F32 = mybir.dt.float32
F32R = mybir.dt.float32r
BF16 = mybir.dt.bfloat16
U32 = mybir.dt.uint32

B, T, IN, OUT, E = 2048, 8, 1024, 4096, 12
NCORES = 8
BS = B // NCORES            # 256 tokens per core (gating shard)
CS = OUT // NCORES          # 512 output columns per core (expert shard)
CAP = 256                   # capacity slots per expert
SLOTS = E * CAP             # 3072
NT = B // 128               # 16 token tiles globally
NTT = BS // 128             # 2 token tiles per core
NKX = IN // 128             # 8 k-tiles over the expert contraction
NMT = CAP // 128            # 2 m-tiles per expert
TH = T // 2                 # 4 t-slices per chunk half
DC = 1024                   # gating d-chunk width
NDC = OUT // DC             # 4 chunks per token tile
NCH = NTT * NDC             # 8 chunks total
ROWW = 4                    # u32 per slot-table row (16B rows)
NA = SLOTS // 128           # 24 slot blocks (m-tiles across all experts)
NWH = 2 * E                 # 24 half-expert W loads
SENTINEL = 3000000000.0     # > B-1 as u32 -> min-merge keeps real ids


def build_kernel(enable_asserts: bool = False):
    nc = bacc.Bacc(
        "TRN2",
        target_bir_lowering=False,
        debug=False,
        enable_asserts=enable_asserts,
        num_devices=NCORES,
    )

    # ---- I/O -------------------------------------------------------------
    t_sh = nc.dram_tensor("t_sh", [BS, T, OUT], F32, kind="ExternalInput")
    x_full = nc.dram_tensor("x_full", [B, IN], F32R, kind="ExternalInput")
    w_sh = nc.dram_tensor("w_sh", [E, IN, CS], F32, kind="ExternalInput")
    b_sh = nc.dram_tensor("b_sh", [1, E * CS], F32, kind="ExternalInput")
    wg_s = nc.dram_tensor("wg_s", [OUT, E], F32, kind="ExternalInput")  # Wg/T
    bg_r = nc.dram_tensor("bg_r", [1, E], F32, kind="ExternalInput")
    ident = nc.dram_tensor("ident", [128, 128], F32, kind="ExternalInput")
    identr = nc.dram_tensor("identr", [128, 128], F32R, kind="ExternalInput")
    lsl = nc.dram_tensor("lsl", [128, 128], F32, kind="ExternalInput")
    bcast16 = nc.dram_tensor("bcast16", [NT, NT * 128], F32, kind="ExternalInput")
    ltmask16 = nc.dram_tensor("ltmask16", [NT, NT * E], F32, kind="ExternalInput")
    iota_rep = nc.dram_tensor("iota_rep", [128, NT * E], F32, kind="ExternalInput")
    tokid4 = nc.dram_tensor("tokid4", [128, NT * ROWW], U32, kind="ExternalInput")

    out_slots = nc.dram_tensor("out_slots", [SLOTS, CS], BF16, kind="ExternalOutput")
    top1_out = nc.dram_tensor("top1_out", [B, 1], U32, kind="ExternalOutput")

    with tile.TileContext(nc) as tc:
        with (
            tc.tile_pool(name="consts", bufs=1) as cpool,
            tc.tile_pool(name="dram", bufs=1, space="DRAM") as dpool,
            tc.tile_pool(name="wf", bufs=2) as wfpool,
            tc.tile_pool(name="wp", bufs=7) as wpool,
            tc.tile_pool(name="gat", bufs=4) as gpool,
            tc.tile_pool(name="gat1", bufs=1) as g1pool,
            # PSUM budget (8 banks): tp x3 + tpg x2 + gps x1 + po x2
            tc.tile_pool(name="gps", bufs=3, space="PSUM") as gpsum,
            tc.tile_pool(name="gpsg", bufs=2, space="PSUM") as gpsumg,
            tc.tile_pool(name="gps1", bufs=1, space="PSUM") as gpsum1,
            tc.tile_pool(name="rout", bufs=1) as r1pool,
            tc.tile_pool(name="mrg", bufs=3) as mpool,
            tc.tile_pool(name="scr", bufs=4) as spool,
            tc.tile_pool(name="xp", bufs=4) as xpool,
            tc.tile_pool(name="op", bufs=3) as opool,
            tc.tile_pool(name="ops", bufs=2, space="PSUM") as opsum,
        ):
            # ---- dummies holding the first two W-f32 slots until the last
            # gating chunks release them (keeps phase-1 HBM for t) ---------
            wdum = []
            for i in range(2):
                dm = wfpool.tile([1, 1], F32, tag="wf", name=f"wdum{i}")
                nc.vector.memset(dm[:], 0.0)
                wdum.append(dm)

            # ---- constants resident in SBUF for the whole kernel ---------
            ident_sb = cpool.tile([128, 128], F32)
            nc.scalar.dma_start(ident_sb[:], ident[:, :])
            identr_sb = cpool.tile([128, 128], F32R)
            nc.scalar.dma_start(identr_sb[:], identr[:, :])
            lsl_sb = cpool.tile([128, 128], F32)
            nc.scalar.dma_start(lsl_sb[:], lsl[:, :])
            bcast16_sb = cpool.tile([NT, NT * 128], F32)
            nc.scalar.dma_start(bcast16_sb[:], bcast16[:, :])
            ltmask16_sb = cpool.tile([NT, NT * E], F32)
            nc.scalar.dma_start(ltmask16_sb[:], ltmask16[:, :])
            iota_rep_sb = cpool.tile([128, NT * E], F32)
            nc.scalar.dma_start(iota_rep_sb[:], iota_rep[:, :])
            tokid4_sb = cpool.tile([128, NT * ROWW], U32)
            nc.scalar.dma_start(tokid4_sb[:], tokid4[:, :])
            ones_sb = cpool.tile([128, 128], F32)
            nc.vector.memset(ones_sb[:], 1.0)
            ones_bf = cpool.tile([1, 128], BF16)
            nc.vector.memset(ones_bf[:], 1.0)
            # Wg/T laid out [128, 32*E]: wg_sb[p, kt*E+e] = Wg[kt*128+p, e]
            wg_sb = cpool.tile([128, (OUT // 128) * E], F32)
            nc.scalar.dma_start(
                wg_sb[:].rearrange("p (k e) -> p k e", e=E),
                wg_s[:, :].rearrange("(k p) e -> p k e", p=128),
            )
            bg_sb = cpool.tile([1, E], F32)
            nc.scalar.dma_start(bg_sb[:], bg_r[:, :])
            bias_bf = cpool.tile([1, E * CS], BF16)
            nc.gpsimd.dma_start(bias_bf[:], b_sh[:, :])
            sent_sb = cpool.tile([128, NA * ROWW], U32)
            nc.vector.memset(sent_sb[:], SENTINEL)

            # DRAM scratch
            top1_loc = dpool.tile([BS, 1], U32, name="t1loc")
            all_top1 = dpool.tile([B, 1], U32, name="allt1")
            warm_in = dpool.tile([1, 1], U32, name="warmi")
            warm_out = dpool.tile([NCORES, 1], U32, name="warmo")
            tabs = [
                dpool.tile([SLOTS, ROWW], U32, name=f"tab{j}") for j in range(NT)
            ]

            # warm-up collective: cross-core barrier + ncfw pre-arm, hidden
            # under the t stream (data content irrelevant)
            nc.scalar.dma_start(warm_in[:, :], sent_sb[0:1, 0:1])
            nc.gpsimd.collective_compute(
                "AllGather",
                mybir.AluOpType.bypass,
                replica_groups=[list(range(NCORES))],
                ins=[warm_in[:].opt()],
                outs=[warm_out[:].opt()],
            )

            # ================= phase 1: gating ============================
            qeng = [nc.scalar, nc.sync]
            ci = 0
            for tt in range(NTT):
                gps = gpsum1.tile([E, 128], F32, tag="gps")
                for dc in range(NDC):
                    # chunk holds t[:, 0:4, :]; the second T-half is folded
                    # in by an accumulate-DMA (first reduce level in-DMA)
                    chunk = gpool.tile([128, TH, DC], F32, tag="tchunk")
                    qeng[ci % 2].dma_start(
                        chunk[:],
                        t_sh[tt * 128 : (tt + 1) * 128, 0:TH,
                             dc * DC : (dc + 1) * DC],
                    )
                    nc.gpsimd.dma_start(
                        chunk[:],
                        t_sh[tt * 128 : (tt + 1) * 128, TH:T,
                             dc * DC : (dc + 1) * DC],
                        accum_op=mybir.AluOpType.add,
                    )
                    # remaining tree levels on DVE -- exact f32
                    cf = chunk[:].rearrange("p t d -> p (t d)")
                    nc.vector.tensor_add(
                        cf[:, 0 : 2 * DC], cf[:, 0 : 2 * DC], cf[:, 2 * DC : 4 * DC]
                    )
                    nc.vector.tensor_add(
                        cf[:, 0:DC], cf[:, 0:DC], cf[:, DC : 2 * DC]
                    )
                    for k in range(DC // 128):
                        kt = dc * (DC // 128) + k
                        ptr = gpsum.tile([128, 128], F32, tag="tp")
                        nc.tensor.transpose(
                            ptr[:],
                            chunk[:, 0, k * 128 : (k + 1) * 128],
                            ident_sb[:, :],
                        )
                        tst = gpool.tile([128, 128], F32, tag="tsT", bufs=4)
                        nc.vector.tensor_copy(tst[:], ptr[:])
                        nc.tensor.matmul(
                            gps[:],
                            lhsT=wg_sb[:, kt * E : (kt + 1) * E],
                            rhs=tst[:],
                            start=(kt == 0),
                            stop=False,
                        )
                    # release a W-f32 slot on the last two chunks
                    if ci >= NCH - 2:
                        di = ci - (NCH - 2)
                        scr = spool.tile([1, 1], F32, tag="scr")
                        nc.vector.tensor_add(
                            scr[:], wdum[di][:], chunk[0:1, 0, 0:1]
                        )
                    ci += 1
                nc.tensor.matmul(
                    gps[:],
                    lhsT=bg_sb[0:1, :],
                    rhs=ones_sb[0:1, 0:128],
                    start=False,
                    stop=True,
                )
                gT_sb = gpool.tile([E, 128], F32, tag="gT")
                nc.vector.tensor_copy(gT_sb[:], gps[:])
                gp = gpsumg.tile([128, E], F32, tag="tpg")
                nc.tensor.transpose(gp[:], gT_sb[:], ident_sb[0:E, 0:E])
                gate_sb = gpool.tile([128, E], F32, tag="gate")
                nc.vector.tensor_copy(gate_sb[:], gp[:])
                mxv = gpool.tile([128, 8], F32, tag="mxv")
                mxi = gpool.tile([128, 8], U32, tag="mxi")
                nc.vector.max_with_indices(mxv[:], mxi[:], gate_sb[:])
                nc.sync.dma_start(
                    top1_loc[tt * 128 : (tt + 1) * 128, :], mxi[:, 0:1]
                )

            # sentinel-init the sub-tables (contiguous; behind the chunks)
            for j in range(NT):
                nc.scalar.dma_start(
                    tabs[j][:, :].rearrange("(p q) n -> p (q n)", p=128),
                    sent_sb[:],
                )

            # ---- W-f32 half-expert stream + DVE bf16 cast pipeline -------
            wfhs = []
            wts = []

            def load_wfh(k):
                wf = wfpool.tile([128, (NKX // 2) * CS], F32, tag="wf",
                                 name=f"wf{k}")
                nc.sync.dma_start(
                    wf[:].rearrange("p (k n) -> p k n", k=NKX // 2),
                    w_sh[k // 2].rearrange("(k p) n -> p k n", p=128)[
                        :, (k % 2) * (NKX // 2) : (k % 2 + 1) * (NKX // 2), :
                    ],
                )
                wfhs.append(wf)

            def cast_wh(k):
                e, h = k // 2, k % 2
                if h == 0:
                    wt = wpool.tile([128, NKX * CS], BF16, tag="wt",
                                    name=f"wt{e}")
                    wts.append(wt)
                half = (NKX // 2) * CS
                nc.vector.tensor_copy(
                    wts[e][:, h * half : (h + 1) * half], wfhs[k][:]
                )

            for k in range(12):
                load_wfh(k)
            for k in range(10):
                cast_wh(k)

            # ================= phase 2: the real AllGather ================
            nc.gpsimd.collective_compute(
                "AllGather",
                mybir.AluOpType.bypass,
                replica_groups=[list(range(NCORES))],
                ins=[top1_loc[:].opt()],
                outs=[all_top1[:].opt()],
            )
            nc.scalar.dma_start(top1_out[:, :], all_top1[:, :])

            # ================= phase 3: slot assignment (batched) =========
            tb_all = r1pool.tile([128, NT], U32)
            nc.scalar.dma_start(
                tb_all[:],
                all_top1[:, :].rearrange("(j p) one -> p (j one)", p=128),
            )
            t1f_all = r1pool.tile([128, NT], F32)
            nc.vector.tensor_copy(t1f_all[:], tb_all[:])
            oh_all = r1pool.tile([128, NT * E], F32)
            nc.vector.tensor_tensor(
                out=oh_all[:].rearrange("p (j e) -> p j e", e=E),
                in0=t1f_all[:, :, None].to_broadcast([128, NT, E]),
                in1=iota_rep_sb[:].rearrange("p (j e) -> p j e", e=E),
                op=mybir.AluOpType.is_equal,
            )
            # replicated per-tile counts in one matmul; tile-base prefix via
            # mask + reduce
            crep = gpsumg.tile([NT, NT * E], F32, tag="tpg")
            nc.tensor.matmul(
                crep[:], lhsT=ones_sb[:, 0:NT], rhs=oh_all[:],
                start=True, stop=True,
            )
            cmask = r1pool.tile([NT, NT * E], F32)
            nc.vector.tensor_mul(cmask[:], crep[:], ltmask16_sb[:])
            b2_sb = r1pool.tile([NT, E], F32)
            nc.vector.reduce_sum(
                b2_sb[:],
                cmask[:].rearrange("p (j e) -> p e j", e=E),
                axis=mybir.AxisListType.X,
            )
            # rank = within-tile exclusive prefix + tile base
            pr1 = gpsumg.tile([128, NT * E], F32, tag="tpg")
            nc.tensor.matmul(
                pr1[:], lhsT=lsl_sb[:], rhs=oh_all[:],
                start=True, stop=True,
            )
            pr2 = gpsumg.tile([128, NT * E], F32, tag="tpg")
            for i in range(NT):
                nc.tensor.matmul(
                    pr2[:, i * E : (i + 1) * E],
                    lhsT=bcast16_sb[:, i * 128 : (i + 1) * 128],
                    rhs=b2_sb[:],
                    start=True,
                    stop=True,
                )
            sel = r1pool.tile([128, NT * E], F32)
            nc.vector.tensor_copy(sel[:], pr1[:])
            nc.vector.tensor_add(sel[:], sel[:], pr2[:])
            nc.vector.tensor_mul(sel[:], sel[:], oh_all[:])
            rank_all = r1pool.tile([128, NT], F32)
            nc.vector.reduce_sum(
                rank_all[:],
                sel[:].rearrange("p (j e) -> p j e", e=E),
                axis=mybir.AxisListType.X,
            )
            # table row r = (rank%128)*NA + top1*NMT + (rank>=128)
            ge = r1pool.tile([128, NT], F32)
            nc.vector.tensor_scalar(
                ge[:], rank_all[:], 128.0, scalar2=None, op0=mybir.AluOpType.is_ge
            )
            rem = r1pool.tile([128, NT], F32)
            nc.vector.tensor_scalar(
                rem[:], ge[:], 128.0, scalar2=None, op0=mybir.AluOpType.mult
            )
            nc.vector.tensor_tensor(
                out=rem[:], in0=rank_all[:], in1=rem[:],
                op=mybir.AluOpType.subtract,
            )
            posf = r1pool.tile([128, NT], F32)
            nc.vector.tensor_scalar(
                posf[:], rem[:], float(NA), scalar2=None, op0=mybir.AluOpType.mult
            )
            t2 = r1pool.tile([128, NT], F32)
            nc.vector.tensor_scalar(
                t2[:], t1f_all[:], float(NMT), scalar2=None,
                op0=mybir.AluOpType.mult,
            )
            nc.vector.tensor_add(posf[:], posf[:], t2[:])
            nc.vector.tensor_add(posf[:], posf[:], ge[:])
            posu = r1pool.tile([128, NT], U32)
            nc.vector.tensor_copy(posu[:], posf[:])
            # 16 independent scatters (no WAW -> pipeline at emission rate)
            for j in range(NT):
                nc.gpsimd.indirect_dma_start(
                    out=tabs[j][:, :],
                    out_offset=IndirectOffsetOnAxis(ap=posu[:, j : j + 1], axis=0),
                    in_=tokid4_sb[:, j * ROWW : (j + 1) * ROWW],
                    in_offset=None,
                    bounds_check=SLOTS - 1,
                    oob_is_err=False,
                )
            # contiguous read-back + min-merge into the slot->token map
            pslice = r1pool.tile([128, NA * ROWW], U32)
            for j in range(NT):
                m = mpool.tile([128, NA * ROWW], U32, tag="mg")
                nc.scalar.dma_start(
                    m[:],
                    tabs[j][:, :].rearrange("(p q) n -> p (q n)", p=128),
                )
                if j == 0:
                    nc.vector.tensor_copy(pslice[:], m[:])
                else:
                    nc.vector.tensor_tensor(
                        out=pslice[:], in0=pslice[:], in1=m[:],
                        op=mybir.AluOpType.min,
                    )

            # ================= phase 4: expert matmul =====================
            def gather_x(e, mt):
                a = e * NMT + mt
                xg = xpool.tile([128, IN], F32R, tag="xg")
                nc.gpsimd.indirect_dma_start(
                    out=xg[:],
                    out_offset=None,
                    in_=x_full[:, :],
                    in_offset=IndirectOffsetOnAxis(
                        ap=pslice[:, a * ROWW : a * ROWW + 1], axis=0
                    ),
                    bounds_check=B - 1,
                    oob_is_err=False,
                )
                return xg

            xgs = {}
            for e in range(2):
                for mt in range(NMT):
                    xgs[(e, mt)] = gather_x(e, mt)

            for e in range(E):
                wt = wts[e]
                for mt in range(NMT):
                    xg = xgs.pop((e, mt))
                    xgT = xpool.tile([128, IN], BF16, tag="xgT")
                    for k in range(NKX):
                        ptx = gpsum.tile([128, 128], F32R, tag="tp")
                        nc.tensor.transpose(
                            ptx[:],
                            xg[:, k * 128 : (k + 1) * 128],
                            identr_sb[:, :],
                        )
                        nc.vector.tensor_copy(
                            xgT[:, k * 128 : (k + 1) * 128], ptx[:]
                        )
                    po = opsum.tile([128, CS], F32, tag="po")
                    for k in range(NKX):
                        nc.tensor.matmul(
                            po[:],
                            lhsT=xgT[:, k * 128 : (k + 1) * 128],
                            rhs=wt[:, k * CS : (k + 1) * CS],
                            start=(k == 0),
                            stop=False,
                        )
                    nc.tensor.matmul(
                        po[:],
                        lhsT=ones_bf[0:1, :],
                        rhs=bias_bf[0:1, e * CS : (e + 1) * CS],
                        start=False,
                        stop=True,
                    )
                    ot = opool.tile([128, CS], BF16, tag="ot")
                    nc.vector.tensor_copy(ot[:], po[:])
                    nc.sync.dma_start(
                        out_slots[(e * NMT + mt) * 128 : (e * NMT + mt + 1) * 128, :],
                        ot[:],
                    )
                # software pipeline: next gathers, W casts, W-f32 loads
                if e + 2 < E:
                    for mt in range(NMT):
                        xgs[(e + 2, mt)] = gather_x(e + 2, mt)
                for k in (2 * e + 10, 2 * e + 11):
                    if k < NWH:
                        cast_wh(k)
                for k in (2 * e + 12, 2 * e + 13):
                    if k < NWH:
                        load_wfh(k)

    nc.compile()
    return nc


def make_in_maps(inputs: dict) -> list[dict]:
    x = np.ascontiguousarray(np.asarray(inputs["x"], dtype=np.float32))
    t = np.ascontiguousarray(np.asarray(inputs["t"], dtype=np.float32))
    W = np.ascontiguousarray(np.asarray(inputs["W"], dtype=np.float32))
    b = np.ascontiguousarray(np.asarray(inputs["b"], dtype=np.float32))
    Wg = np.ascontiguousarray(np.asarray(inputs["Wg"], dtype=np.float32))
    bg = np.ascontiguousarray(np.asarray(inputs["bg"], dtype=np.float32))

    x2 = np.ascontiguousarray(x[:, 0, :])                       # [B, IN]
    ident = np.eye(128, dtype=np.float32)
    lsl = np.triu(np.ones((128, 128), np.float32), k=1)          # lsl[r,c]=1 iff r<c
    bcast16 = np.zeros((NT, NT * 128), np.float32)
    for i in range(NT):
        bcast16[i, i * 128 : (i + 1) * 128] = 1.0
    ltmask16 = np.zeros((NT, NT * E), np.float32)
    for j in range(NT):
        for jp in range(j):
            ltmask16[j, jp * E : (jp + 1) * E] = 1.0
    iota_rep = np.tile(np.arange(E, dtype=np.float32)[None, :], (128, NT))
    tokid4 = np.zeros((128, NT * ROWW), np.uint32)
    for j in range(NT):
        tokid4[:, j * ROWW : (j + 1) * ROWW] = (
            j * 128 + np.arange(128, dtype=np.uint32)[:, None]
        )

    in_maps = []
    for c in range(NCORES):
        cs = slice(c * CS, (c + 1) * CS)
        in_maps.append({
            "t_sh": np.ascontiguousarray(t[c * BS : (c + 1) * BS]),
            "x_full": x2,
            "w_sh": np.ascontiguousarray(W[:, :, cs]),
            "b_sh": np.ascontiguousarray(b[:, cs]).reshape(1, E * CS),
            "wg_s": np.ascontiguousarray(Wg / float(T)),
            "bg_r": bg.reshape(1, E),
            "ident": ident,
            "identr": ident,
            "lsl": lsl,
            "bcast16": bcast16,
            "ltmask16": ltmask16,
            "iota_rep": iota_rep,
            "tokid4": tokid4,
        })
    return in_maps


def assemble_output(per_core_results: list[dict]) -> np.ndarray:
    top1 = np.asarray(per_core_results[0]["top1_out"]).reshape(B).astype(np.int64)
    rank = np.zeros(B, dtype=np.int64)
    counts = np.zeros(E, dtype=np.int64)
    for g in range(B):
        e = top1[g]
        rank[g] = counts[e]
        counts[e] += 1
    assert counts.max() <= CAP, f"expert overflow: {counts}"
    slot = top1 * CAP + rank
    out = np.empty((B, 1, OUT), dtype=np.float32)
    for c in range(NCORES):
        osl = np.asarray(per_core_results[c]["out_slots"]).astype(np.float32)
        out[:, 0, c * CS : (c + 1) * CS] = osl[slot]
    return out


_NC_CACHE = {}


def kernel(**inputs) -> np.ndarray:
    if "nc" not in _NC_CACHE:
        _NC_CACHE["nc"] = build_kernel()
    nc = _NC_CACHE["nc"]
    in_maps = make_in_maps(inputs)
    res = run_bass_kernel_spmd(nc, in_maps, core_ids=list(range(NCORES)))
    return assemble_output(res.results)


# revision 37
# speedup vs baseline: 1.4117x; 1.0227x over previous
"""Top-1 MoE mapper kernel for Trainium2, SPMD over 8 NeuronCores.

Problem (hardcoded shapes):
  x  [2048, 1, 1024] f32   token inputs
  t  [2048, 8, 4096] f32   gating context
  W  [12, 1024, 4096] f32  expert weights
  b  [12, 4096] f32        expert biases
  Wg [4096, 12] f32        gate weights
  bg [12] f32              gate bias
  out[b] = x[b] @ W[argmax(t[b].mean(T) @ Wg + bg)] + b[...]  -> [2048, 1, 4096]

Strategy (v7):
  - Gating data-parallel over B. Each 4MB t-chunk is fetched as TWO 2MB
    halves (t-slices 0:4 and 4:8) on the alternating HWDGE queues; the
    first tree-reduce level (A += B) is split across GpSimd and DVE so
    neither engine paces the stream (accumulate-DMA was tried and only
    reaches ~130GB/s). PE transposes, f32 gate matmul, argmax unchanged.
    Gating is f32 end-to-end so the device top-1 matches the reference.
  - A zero-cost WARM-UP AllGather at the top of the gpsimd FIFO acts as a
    cross-core barrier and pre-arms the collective firmware while phase 1
    streams; the REAL AllGather (after the reduce ops) then pays less of
    the ~40us arm/skew latency.
  - W streams as f32 HALF-EXPERT loads on the HWDGE queues, first two held
    back by dummy-slot WARs released by the last gating chunks; all
    f32->bf16 W casts on DVE into a 7-slot bf16 pool.
  - Routing fully batched (single-op one-hots / counts matmul / mask-reduce
    tile bases / lsl rank matmul / single-op rank reduce). The slot->token
    scatter goes to 16 INDEPENDENT sentinel-initialized sub-tables stored
    PARTITION-MAJOR (row r = p*24 + a for slot a*128+p) so init and
    read-back are contiguous DMAs and the 16 scatters pipeline at emission
    rate; tables min-merge on DVE into the SBUF slot->token map. Padded
    slots keep the sentinel and drop their gather traffic.
  - Expert matmul output-column-parallel: per m-tile a 128-row indirect
    gather of x (the only gpsimd-queue work in phase 4), PE transposes,
    bf16 matmuls (N=512), bf16 bias via K=1 matmul, bf16 results written
    contiguously in slot order.
  - Host unpermutes slots -> tokens using the device-computed top-1 ids
    (pure data movement; all routing math happens on device).
"""

import numpy as np

import concourse.bass as bass
import concourse.bacc as bacc
import concourse.mybir as mybir
import concourse.tile as tile
from concourse.bass import IndirectOffsetOnAxis
from concourse.bass_utils import run_bass_kernel_spmd

F32 = mybir.dt.float32
F32R = mybir.dt.float32r
BF16 = mybir.dt.bfloat16
U32 = mybir.dt.uint32

B, T, IN, OUT, E = 2048, 8, 1024, 4096, 12
NCORES = 8
BS = B // NCORES            # 256 tokens per core (gating shard)
CS = OUT // NCORES          # 512 output columns per core (expert shard)
CAP = 256                   # capacity slots per expert
SLOTS = E * CAP             # 3072
NT = B // 128               # 16 token tiles globally
NTT = BS // 128             # 2 token tiles per core
NKX = IN // 128             # 8 k-tiles over the expert contraction
NMT = CAP // 128            # 2 m-tiles per expert
TH = T // 2                 # 4 t-slices per chunk half
DC = 1024                   # gating d-chunk width
NDC = OUT // DC             # 4 chunks per token tile
NCH = NTT * NDC             # 8 chunks total
ROWW = 4                    # u32 per slot-table row (16B rows)
NA = SLOTS // 128           # 24 slot blocks (m-tiles across all experts)
NWH = 2 * E                 # 24 half-expert W loads
SENTINEL = 3000000000.0     # > B-1 as u32 -> min-merge keeps real ids


def build_kernel(enable_asserts: bool = False):
    nc = bacc.Bacc(
        "TRN2",
        target_bir_lowering=False,
        debug=False,
        enable_asserts=enable_asserts,
        num_devices=NCORES,
    )

    # ---- I/O -------------------------------------------------------------
    t_sh = nc.dram_tensor("t_sh", [BS, T, OUT], F32, kind="ExternalInput")
    x_full = nc.dram_tensor("x_full", [B, IN], F32R, kind="ExternalInput")
    w_sh = nc.dram_tensor("w_sh", [E, IN, CS], F32, kind="ExternalInput")
    b_sh = nc.dram_tensor("b_sh", [1, E * CS], F32, kind="ExternalInput")
    wg_s = nc.dram_tensor("wg_s", [OUT, E], F32, kind="ExternalInput")  # Wg/T
    bg_r = nc.dram_tensor("bg_r", [1, E], F32, kind="ExternalInput")
    ident = nc.dram_tensor("ident", [128, 128], F32, kind="ExternalInput")
    identr = nc.dram_tensor("identr", [128, 128], F32R, kind="ExternalInput")
    lsl = nc.dram_tensor("lsl", [128, 128], F32, kind="ExternalInput")
    iota_rep = nc.dram_tensor("iota_rep", [128, NT * E], F32, kind="ExternalInput")
    tokid4 = nc.dram_tensor("tokid4", [128, NT * ROWW], U32, kind="ExternalInput")

    out_slots = nc.dram_tensor("out_slots", [SLOTS, CS], BF16, kind="ExternalOutput")
    top1_out = nc.dram_tensor("top1_out", [B, 1], U32, kind="ExternalOutput")

    with tile.TileContext(nc) as tc:
        with (
            tc.tile_pool(name="consts", bufs=1) as cpool,
            tc.tile_pool(name="dram", bufs=1, space="DRAM") as dpool,
            tc.tile_pool(name="wf", bufs=2) as wfpool,
            tc.tile_pool(name="wp", bufs=6) as wpool,
            tc.tile_pool(name="gat", bufs=3) as gpool,
            tc.tile_pool(name="gat1", bufs=1) as g1pool,
            # PSUM budget (8 banks): tp x3 + tpg x2 + gps x1 + po x2
            tc.tile_pool(name="gps", bufs=3, space="PSUM") as gpsum,
            tc.tile_pool(name="gpsg", bufs=2, space="PSUM") as gpsumg,
            tc.tile_pool(name="gps1", bufs=1, space="PSUM") as gpsum1,
            tc.tile_pool(name="rout", bufs=1) as r1pool,
            tc.tile_pool(name="mrg", bufs=2) as mpool,
            tc.tile_pool(name="scr", bufs=4) as spool,
            tc.tile_pool(name="xp", bufs=3) as xpool,
            tc.tile_pool(name="op", bufs=2) as opool,
            tc.tile_pool(name="ops", bufs=2, space="PSUM") as opsum,
        ):
            # ---- dummies holding the first two W-f32 slots until the last
            # gating chunks release them (keeps phase-1 HBM for t) ---------
            wdum = []
            for i in range(2):
                dm = wfpool.tile([1, 1], F32, tag="wf", name=f"wdum{i}")
                nc.vector.memset(dm[:], 0.0)
                wdum.append(dm)

            # ---- constants resident in SBUF for the whole kernel ---------
            ident_sb = cpool.tile([128, 128], F32)
            nc.scalar.dma_start(ident_sb[:], ident[:, :])
            identr_sb = cpool.tile([128, 128], F32R)
            nc.scalar.dma_start(identr_sb[:], identr[:, :])
            lsl_sb = cpool.tile([128, 128], F32)
            nc.scalar.dma_start(lsl_sb[:], lsl[:, :])
            iota_rep_sb = cpool.tile([128, NT * E], F32)
            nc.scalar.dma_start(iota_rep_sb[:], iota_rep[:, :])
            tokid4_sb = cpool.tile([128, NT * ROWW], U32)
            nc.scalar.dma_start(tokid4_sb[:], tokid4[:, :])
            ones_sb = cpool.tile([128, 128], F32)
            nc.vector.memset(ones_sb[:], 1.0)
            ones_bf = cpool.tile([1, 128], BF16)
            nc.vector.memset(ones_bf[:], 1.0)
            # Wg/T laid out [128, 32*E]: wg_sb[p, kt*E+e] = Wg[kt*128+p, e]
            wg_sb = cpool.tile([128, (OUT // 128) * E], F32)
            nc.scalar.dma_start(
                wg_sb[:].rearrange("p (k e) -> p k e", e=E),
                wg_s[:, :].rearrange("(k p) e -> p k e", p=128),
            )
            bg_sb = cpool.tile([1, E], F32)
            nc.scalar.dma_start(bg_sb[:], bg_r[:, :])
            bias_bf = cpool.tile([1, E * CS], BF16)
            nc.gpsimd.dma_start(bias_bf[:], b_sh[:, :])
            sent_sb = cpool.tile([128, NA * ROWW], U32)
            nc.vector.memset(sent_sb[:], SENTINEL)

            # DRAM scratch
            top1_loc = dpool.tile([BS, 1], U32, name="t1loc")
            all_top1 = dpool.tile([B, 1], U32, name="allt1")
            warm_in = dpool.tile([1, 1], U32, name="warmi")
            warm_out = dpool.tile([NCORES, 1], U32, name="warmo")
            tabs = [
                dpool.tile([SLOTS, ROWW], U32, name=f"tab{j}") for j in range(NT)
            ]

            # warm-up collective: cross-core barrier + ncfw pre-arm, hidden
            # under the t stream (data content irrelevant)
            nc.scalar.dma_start(warm_in[:, :], sent_sb[0:1, 0:1])
            nc.gpsimd.collective_compute(
                "AllGather",
                mybir.AluOpType.bypass,
                replica_groups=[list(range(NCORES))],
                ins=[warm_in[:].opt()],
                outs=[warm_out[:].opt()],
            )

            # ================= phase 1: gating ============================
            qeng = [nc.scalar, nc.sync]
            ci = 0
            for tt in range(NTT):
                gps = gpsum1.tile([E, 128], F32, tag="gps")
                for dc in range(NDC):
                    # two plain 2MB halves on the alternating HWDGE queues
                    chunk = gpool.tile([128, TH, DC], F32, tag="tchunk")
                    chkb = gpool.tile([128, TH, DC], F32, tag="tchunkb")
                    qeng[ci % 2].dma_start(
                        chunk[:],
                        t_sh[tt * 128 : (tt + 1) * 128, 0:TH,
                             dc * DC : (dc + 1) * DC],
                    )
                    qeng[(ci + 1) % 2].dma_start(
                        chkb[:],
                        t_sh[tt * 128 : (tt + 1) * 128, TH:T,
                             dc * DC : (dc + 1) * DC],
                    )
                    # tree-reduce: level 1 split GpSimd/DVE, rest on DVE
                    cf = chunk[:].rearrange("p t d -> p (t d)")
                    cb = chkb[:].rearrange("p t d -> p (t d)")
                    nc.gpsimd.tensor_add(
                        cf[:, 0 : 2 * DC], cf[:, 0 : 2 * DC], cb[:, 0 : 2 * DC]
                    )
                    nc.vector.tensor_add(
                        cf[:, 2 * DC : 4 * DC],
                        cf[:, 2 * DC : 4 * DC],
                        cb[:, 2 * DC : 4 * DC],
                    )
                    nc.vector.tensor_add(
                        cf[:, 0 : 2 * DC], cf[:, 0 : 2 * DC], cf[:, 2 * DC : 4 * DC]
                    )
                    nc.vector.tensor_add(
                        cf[:, 0:DC], cf[:, 0:DC], cf[:, DC : 2 * DC]
                    )
                    for k in range(DC // 128):
                        kt = dc * (DC // 128) + k
                        ptr = gpsum.tile([128, 128], F32, tag="tp")
                        nc.tensor.transpose(
                            ptr[:],
                            chunk[:, 0, k * 128 : (k + 1) * 128],
                            ident_sb[:, :],
                        )
                        tst = gpool.tile([128, 128], F32, tag="tsT", bufs=3)
                        nc.vector.tensor_copy(tst[:], ptr[:])
                        nc.tensor.matmul(
                            gps[:],
                            lhsT=wg_sb[:, kt * E : (kt + 1) * E],
                            rhs=tst[:],
                            start=(kt == 0),
                            stop=False,
                        )
                    # release a W-f32 slot on the last two chunks
                    if ci >= NCH - 2:
                        di = ci - (NCH - 2)
                        scr = spool.tile([1, 1], F32, tag="scr")
                        nc.vector.tensor_add(
                            scr[:], wdum[di][:], chunk[0:1, 0, 0:1]
                        )
                    ci += 1
                nc.tensor.matmul(
                    gps[:],
                    lhsT=bg_sb[0:1, :],
                    rhs=ones_sb[0:1, 0:128],
                    start=False,
                    stop=True,
                )
                gT_sb = gpool.tile([E, 128], F32, tag="gT")
                nc.vector.tensor_copy(gT_sb[:], gps[:])
                gp = gpsumg.tile([128, E], F32, tag="tpg")
                nc.tensor.transpose(gp[:], gT_sb[:], ident_sb[0:E, 0:E])
                gate_sb = gpool.tile([128, E], F32, tag="gate")
                nc.vector.tensor_copy(gate_sb[:], gp[:])
                mxv = gpool.tile([128, 8], F32, tag="mxv")
                mxi = gpool.tile([128, 8], U32, tag="mxi")
                nc.vector.max_with_indices(mxv[:], mxi[:], gate_sb[:])
                nc.sync.dma_start(
                    top1_loc[tt * 128 : (tt + 1) * 128, :], mxi[:, 0:1]
                )

            # sentinel-init the sub-tables (contiguous; behind the chunks)
            for j in range(NT):
                nc.scalar.dma_start(
                    tabs[j][:, :].rearrange("(p q) n -> p (q n)", p=128),
                    sent_sb[:],
                )

            # ---- W-f32 half-expert stream + DVE bf16 cast pipeline -------
            wfhs = []
            wts = []

            def load_wfh(k):
                wf = wfpool.tile([128, (NKX // 2) * CS], F32, tag="wf",
                                 name=f"wf{k}")
                nc.sync.dma_start(
                    wf[:].rearrange("p (k n) -> p k n", k=NKX // 2),
                    w_sh[k // 2].rearrange("(k p) n -> p k n", p=128)[
                        :, (k % 2) * (NKX // 2) : (k % 2 + 1) * (NKX // 2), :
                    ],
                )
                wfhs.append(wf)

            def cast_wh(k):
                e, h = k // 2, k % 2
                if h == 0:
                    wt = wpool.tile([128, NKX * CS], BF16, tag="wt",
                                    name=f"wt{e}")
                    wts.append(wt)
                half = (NKX // 2) * CS
                nc.vector.tensor_copy(
                    wts[e][:, h * half : (h + 1) * half], wfhs[k][:]
                )

            for k in range(12):
                load_wfh(k)
            for k in range(10):
                cast_wh(k)

            # ================= phase 2: the real AllGather ================
            nc.gpsimd.collective_compute(
                "AllGather",
                mybir.AluOpType.bypass,
                replica_groups=[list(range(NCORES))],
                ins=[top1_loc[:].opt()],
                outs=[all_top1[:].opt()],
            )
            nc.scalar.dma_start(top1_out[:, :], all_top1[:, :])

            # ================= phase 3: slot assignment (batched) =========
            tb_all = r1pool.tile([128, NT], U32)
            nc.scalar.dma_start(
                tb_all[:],
                all_top1[:, :].rearrange("(j p) one -> p (j one)", p=128),
            )
            t1f_all = r1pool.tile([128, NT], F32)
            nc.vector.tensor_copy(t1f_all[:], tb_all[:])
            oh_all = r1pool.tile([128, NT * E], F32)
            nc.vector.tensor_tensor(
                out=oh_all[:].rearrange("p (j e) -> p j e", e=E),
                in0=t1f_all[:, :, None].to_broadcast([128, NT, E]),
                in1=iota_rep_sb[:].rearrange("p (j e) -> p j e", e=E),
                op=mybir.AluOpType.is_equal,
            )
            # per-tile expert counts -> one psum row [1, NT*E]
            pcnt = gpsumg.tile([1, NT * E], F32, tag="tpg")
            nc.tensor.matmul(
                pcnt[:], lhsT=ones_sb[:, 0:1], rhs=oh_all[:],
                start=True, stop=True,
            )
            # exclusive tile-base prefix along the 16 tile-blocks: tiny
            # single-partition Hillis-Steele cumsum, then broadcast to all
            # partitions on gpsimd
            pfx = [r1pool.tile([1, NT * E], F32, name=f"pfx{s}") for s in range(2)]
            nc.vector.memset(pfx[0][:], 0.0)
            nc.vector.tensor_copy(pfx[0][0:1, E:], pcnt[0:1, 0 : (NT - 1) * E])
            cur = 0
            for k in (1, 2, 4, 8):
                nxt = 1 - cur
                nc.vector.tensor_copy(
                    pfx[nxt][0:1, 0 : k * E], pfx[cur][0:1, 0 : k * E]
                )
                nc.vector.tensor_add(
                    pfx[nxt][0:1, k * E :],
                    pfx[cur][0:1, k * E :],
                    pfx[cur][0:1, 0 : (NT - k) * E],
                )
                cur = nxt
            b2bc = r1pool.tile([128, NT * E], F32)
            nc.gpsimd.partition_broadcast(b2bc[:], pfx[cur][0:1, :])
            # rank = within-tile exclusive prefix + tile base
            pr1 = gpsumg.tile([128, NT * E], F32, tag="tpg")
            nc.tensor.matmul(
                pr1[:], lhsT=lsl_sb[:], rhs=oh_all[:],
                start=True, stop=True,
            )
            sel = r1pool.tile([128, NT * E], F32)
            nc.vector.tensor_copy(sel[:], pr1[:])
            nc.vector.tensor_add(sel[:], sel[:], b2bc[:])
            nc.vector.tensor_mul(sel[:], sel[:], oh_all[:])
            rank_all = r1pool.tile([128, NT], F32)
            nc.vector.reduce_sum(
                rank_all[:],
                sel[:].rearrange("p (j e) -> p j e", e=E),
                axis=mybir.AxisListType.X,
            )
            # table row r = (rank%128)*NA + top1*NMT + (rank>=128)
            ge = r1pool.tile([128, NT], F32)
            nc.vector.tensor_scalar(
                ge[:], rank_all[:], 128.0, scalar2=None, op0=mybir.AluOpType.is_ge
            )
            rem = r1pool.tile([128, NT], F32)
            nc.vector.tensor_scalar(
                rem[:], ge[:], 128.0, scalar2=None, op0=mybir.AluOpType.mult
            )
            nc.vector.tensor_tensor(
                out=rem[:], in0=rank_all[:], in1=rem[:],
                op=mybir.AluOpType.subtract,
            )
            posf = r1pool.tile([128, NT], F32)
            nc.vector.tensor_scalar(
                posf[:], rem[:], float(NA), scalar2=None, op0=mybir.AluOpType.mult
            )
            t2 = r1pool.tile([128, NT], F32)
            nc.vector.tensor_scalar(
                t2[:], t1f_all[:], float(NMT), scalar2=None,
                op0=mybir.AluOpType.mult,
            )
            nc.vector.tensor_add(posf[:], posf[:], t2[:])
            nc.vector.tensor_add(posf[:], posf[:], ge[:])
            posu = r1pool.tile([128, NT], U32)
            nc.vector.tensor_copy(posu[:], posf[:])
            # 16 independent scatters (no WAW -> pipeline at emission rate)
            for j in range(NT):
                nc.gpsimd.indirect_dma_start(
                    out=tabs[j][:, :],
                    out_offset=IndirectOffsetOnAxis(ap=posu[:, j : j + 1], axis=0),
                    in_=tokid4_sb[:, j * ROWW : (j + 1) * ROWW],
                    in_offset=None,
                    bounds_check=SLOTS - 1,
                    oob_is_err=False,
                )
            # contiguous read-back + min-merge into the slot->token map
            pslice = r1pool.tile([128, NA * ROWW], U32)
            for j in range(NT):
                m = mpool.tile([128, NA * ROWW], U32, tag="mg")
                nc.scalar.dma_start(
                    m[:],
                    tabs[j][:, :].rearrange("(p q) n -> p (q n)", p=128),
                )
                if j == 0:
                    nc.vector.tensor_copy(pslice[:], m[:])
                else:
                    nc.vector.tensor_tensor(
                        out=pslice[:], in0=pslice[:], in1=m[:],
                        op=mybir.AluOpType.min,
                    )

            # ================= phase 4: expert matmul =====================
            def gather_x(e, mt):
                a = e * NMT + mt
                xg = xpool.tile([128, IN], F32R, tag="xg")
                nc.gpsimd.indirect_dma_start(
                    out=xg[:],
                    out_offset=None,
                    in_=x_full[:, :],
                    in_offset=IndirectOffsetOnAxis(
                        ap=pslice[:, a * ROWW : a * ROWW + 1], axis=0
                    ),
                    bounds_check=B - 1,
                    oob_is_err=False,
                )
                return xg

            xgs = {}
            for e in range(2):
                for mt in range(NMT):
                    xgs[(e, mt)] = gather_x(e, mt)

            for e in range(E):
                wt = wts[e]
                for mt in range(NMT):
                    xg = xgs.pop((e, mt))
                    xgT = xpool.tile([128, IN], BF16, tag="xgT")
                    for k in range(NKX):
                        ptx = gpsum.tile([128, 128], F32R, tag="tp")
                        nc.tensor.transpose(
                            ptx[:],
                            xg[:, k * 128 : (k + 1) * 128],
                            identr_sb[:, :],
                        )
                        nc.vector.tensor_copy(
                            xgT[:, k * 128 : (k + 1) * 128], ptx[:]
                        )
                    po = opsum.tile([128, CS], F32, tag="po")
                    for k in range(NKX):
                        nc.tensor.matmul(
                            po[:],
                            lhsT=xgT[:, k * 128 : (k + 1) * 128],
                            rhs=wt[:, k * CS : (k + 1) * CS],
                            start=(k == 0),
                            stop=False,
                        )
                    nc.tensor.matmul(
                        po[:],
                        lhsT=ones_bf[0:1, :],
                        rhs=bias_bf[0:1, e * CS : (e + 1) * CS],
                        start=False,
                        stop=True,
                    )
                    ot = opool.tile([128, CS], BF16, tag="ot")
                    nc.vector.tensor_copy(ot[:], po[:])
                    nc.sync.dma_start(
                        out_slots[(e * NMT + mt) * 128 : (e * NMT + mt + 1) * 128, :],
                        ot[:],
                    )
                # software pipeline: next gathers, W casts, W-f32 loads
                if e + 2 < E:
                    for mt in range(NMT):
                        xgs[(e + 2, mt)] = gather_x(e + 2, mt)
                for k in (2 * e + 10, 2 * e + 11):
                    if k < NWH:
                        cast_wh(k)
                for k in (2 * e + 12, 2 * e + 13):
                    if k < NWH:
                        load_wfh(k)

    nc.compile()
    return nc


def make_in_maps(inputs: dict) -> list[dict]:
    x = np.ascontiguousarray(np.asarray(inputs["x"], dtype=np.float32))
    t = np.ascontiguousarray(np.asarray(inputs["t"], dtype=np.float32))
    W = np.ascontiguousarray(np.asarray(inputs["W"], dtype=np.float32))
    b = np.ascontiguousarray(np.asarray(inputs["b"], dtype=np.float32))
    Wg = np.ascontiguousarray(np.asarray(inputs["Wg"], dtype=np.float32))
    bg = np.ascontiguousarray(np.asarray(inputs["bg"], dtype=np.float32))

    x2 = np.ascontiguousarray(x[:, 0, :])                       # [B, IN]
    ident = np.eye(128, dtype=np.float32)
    lsl = np.triu(np.ones((128, 128), np.float32), k=1)          # lsl[r,c]=1 iff r<c
    iota_rep = np.tile(np.arange(E, dtype=np.float32)[None, :], (128, NT))
    tokid4 = np.zeros((128, NT * ROWW), np.uint32)
    for j in range(NT):
        tokid4[:, j * ROWW : (j + 1) * ROWW] = (
            j * 128 + np.arange(128, dtype=np.uint32)[:, None]
        )

    in_maps = []
    for c in range(NCORES):
        cs = slice(c * CS, (c + 1) * CS)
        in_maps.append({
            "t_sh": np.ascontiguousarray(t[c * BS : (c + 1) * BS]),
            "x_full": x2,
            "w_sh": np.ascontiguousarray(W[:, :, cs]),
            "b_sh": np.ascontiguousarray(b[:, cs]).reshape(1, E * CS),
            "wg_s": np.ascontiguousarray(Wg / float(T)),
            "bg_r": bg.reshape(1, E),
            "ident": ident,
            "identr": ident,
            "lsl": lsl,
            "iota_rep": iota_rep,
            "tokid4": tokid4,
        })
    return in_maps


def assemble_output(per_core_results: list[dict]) -> np.ndarray:
    top1 = np.asarray(per_core_results[0]["top1_out"]).reshape(B).astype(np.int64)
    rank = np.zeros(B, dtype=np.int64)
    counts = np.zeros(E, dtype=np.int64)
    for g in range(B):
        e = top1[g]
        rank[g] = counts[e]
        counts[e] += 1
    assert counts.max() <= CAP, f"expert overflow: {counts}"
    slot = top1 * CAP + rank
    out = np.empty((B, 1, OUT), dtype=np.float32)
    for c in range(NCORES):
        osl = np.asarray(per_core_results[c]["out_slots"]).astype(np.float32)
        out[:, 0, c * CS : (c + 1) * CS] = osl[slot]
    return out


_NC_CACHE = {}


def kernel(**inputs) -> np.ndarray:
    if "nc" not in _NC_CACHE:
        _NC_CACHE["nc"] = build_kernel()
    nc = _NC_CACHE["nc"]
    in_maps = make_in_maps(inputs)
    res = run_bass_kernel_spmd(nc, in_maps, core_ids=list(range(NCORES)))
    return assemble_output(res.results)
